# revision 22
# baseline (speedup 1.0000x reference)
"""BiGCN (two fused GCNConv + graph mean-pool + FC + log_softmax) on 8 trn2 cores.

Strategy (graph/data parallel, partitioned by destination node range):
  - core c owns nodes [c*NSH, (c+1)*NSH) as edge destinations
  - host sorts edges into per-(dst-tile, table-bank) cells, padded to 128-slot
    chunks with -1 indices (skipped by the gather HW); per-core real counts
    are fed through gpsimd registers so pad slots cost no DMA descriptors
  - the bf16 Hn table is built as 4 bank stripes, each AllGathered separately
    so bank-0 gathers start while later banks are still being produced
  - device: degree count via one-hot matmul -> dinv = 1/sqrt(deg+1)
            Hn = (x @ [W_td | W_bu]) * dinv  (bf16), 4x AllGather -> banks
            dma_gather Hn[src] rows per cell, one-hot matmul scatter into PSUM
            out[d] = dinv[d]*(sum + Hn[d]) + b ; feat = [relu(td),td,relu(bu),bu]
            graph pooling via one-hot matmul, indirect-scatter + AllReduce,
            FC + log_softmax computed replicated on every core.
  - one-hot matrices are built in one DVE tensor_tensor per tile using
    broadcast access patterns (iota row vs per-slot dst-local values).
"""

import math

import numpy as np
import ml_dtypes

import concourse.bass as bass
import concourse.bacc as bacc
import concourse.mybir as mybir
import concourse.tile as tile
from concourse.bass import IndirectOffsetOnAxis
from concourse.bass_utils import run_bass_kernel_spmd
from concourse.library_config import mlp as mlp_lib

BF16 = mybir.dt.bfloat16
F32 = mybir.dt.float32
I16 = mybir.dt.int16
I32 = mybir.dt.int32
AF = mybir.ActivationFunctionType
ALU = mybir.AluOpType
NPBF = ml_dtypes.bfloat16

P = 128  # partitions / tile height


def _split_even(n, k):
    base = n // k
    rem = n % k
    return [base + (1 if i < rem else 0) for i in range(k)]


class Cfg:
    def __init__(self, n_nodes, n_graphs, n_cores, banks, in_f, hid_f, out_f):
        assert n_nodes % n_cores == 0
        self.N = n_nodes
        self.G = n_graphs
        self.NC = n_cores
        self.NSH = n_nodes // n_cores  # nodes per core
        self.T = math.ceil(self.NSH / P)  # dst tiles per core
        self.NSH_P = self.T * P  # padded shard rows
        self.BANKS = min(banks, self.T)
        # bank k holds the stripe of tiles [qt_start[k], qt_start[k+1]) from
        # every core: bank rows = NC * qrows[k]
        self.QT = _split_even(self.T, self.BANKS)  # tiles per bank stripe
        self.QSTART = np.concatenate([[0], np.cumsum(self.QT)])  # tile starts
        self.QROWS = [q * P for q in self.QT]
        for k in range(self.BANKS):
            assert self.NC * self.QROWS[k] <= 32767, "bank idx must fit int16"
        self.IN_F = in_f
        self.HID = hid_f
        self.FW = 2 * hid_f
        assert self.FW == P and in_f == P
        self.OUT_F = out_f
        self.FEAT = 4 * hid_f
        self.GB = math.ceil(self.G / P)
        self.PART_ROWS = (self.G + 2 * P + P - 1) // P * P


def host_prep(cfg, x, edge_index, batch):
    """Build per-core edge grids + constants. Returns (meta, per_core_inputs)."""
    c = cfg
    src = edge_index[0].astype(np.int64)
    dst = edge_index[1].astype(np.int64)
    assert src.min() >= 0 and src.max() < c.N and dst.min() >= 0 and dst.max() < c.N

    # host-side degree normalization (in-degree incl self-loop)
    deg = np.bincount(dst, minlength=c.N).astype(np.float64) + 1.0
    dinv_all = (1.0 / np.sqrt(deg)).astype(np.float32)

    # host-side graph-size reciprocals for mean pooling
    cnt = np.maximum(np.bincount(np.asarray(batch), minlength=c.G), 1)
    cinv = np.zeros(c.GB * P, dtype=np.float32)
    cinv[: c.G] = 1.0 / cnt.astype(np.float64)
    cinvT = cinv.reshape(c.GB, P).T.copy()  # [P, GB]

    qstart_rows = c.QSTART[:-1] * P  # local row where each bank stripe starts
    sc = src // c.NSH  # owner core of src
    so = src % c.NSH  # local row of src
    stile = so // P
    bank = np.searchsorted(c.QSTART[1:], stile, side="right")
    lidx = sc * np.asarray(c.QROWS)[bank] + (so - qstart_rows[bank])

    owner = dst // c.NSH
    tloc = (dst % c.NSH) // P
    dl = ((dst % c.NSH) % P).astype(np.int64)

    ncell = c.NC * c.T * c.BANKS
    cell = (owner * c.T + tloc) * c.BANKS + bank
    order = np.argsort(cell, kind="stable")
    cell_s = cell[order]
    lidx_s = lidx[order]
    dl_s = dl[order]
    counts = np.bincount(cell_s, minlength=ncell).reshape(c.NC, c.T, c.BANKS)
    starts = np.zeros(ncell + 1, dtype=np.int64)
    np.cumsum(counts.reshape(-1), out=starts[1:])

    Bmat = (-(-counts // P)).max(axis=0)  # [T, BANKS] chunks per cell
    GCH = int(Bmat.sum())
    ECOLS = GCH * 8

    g_base = np.empty(c.NC, dtype=np.int64)
    for cc in range(c.NC):
        b = batch[cc * c.NSH : (cc + 1) * c.NSH]
        g_base[cc] = int(b[0])
        assert int(b[-1]) - int(b[0]) < 2 * P, "graph span exceeds 2 blocks"

    # chunk columns ordered (t, j, q); shared by dl, eidx and the gather seq
    dl_col = {}
    col = 0
    cells = []  # active cells in order
    for t in range(c.T):
        for j in range(c.BANKS):
            dl_col[(t, j)] = col
            if Bmat[t, j]:
                cells.append((t, j))
            col += int(Bmat[t, j])
    NG = max(len(cells), 1)

    per_core = []
    for cc in range(c.NC):
        eidx = np.zeros((P, max(ECOLS, 8)), dtype=np.int16)
        dlh = np.full((P, max(GCH, 1)), 200.0, dtype=np.float32)
        nreal = np.zeros((1, NG), dtype=np.int32)
        for gi, (t, j) in enumerate(cells):
            B = int(Bmat[t, j])
            ci = (cc * c.T + t) * c.BANKS + j
            s0, s1 = starts[ci], starts[ci + 1]
            n = int(s1 - s0)
            slots = B * P
            li = np.full(slots, -1, dtype=np.int64)
            dv = np.full(slots, 200.0, dtype=np.float64)
            li[:n] = lidx_s[s0:s1]
            dv[:n] = dl_s[s0:s1]
            if n == 0:
                li[0] = 0  # keep >=1 real idx (sim/ucode edge case)
                n = 1
            nreal[0, gi] = n
            w = li.reshape(slots // 16, 16).T.astype(np.int16)
            cb = dl_col[(t, j)]
            eidx[:, cb * 8 : cb * 8 + B * 8] = np.tile(w, (8, 1))
            dlh[:, cb : cb + B] = dv.reshape(B, P).T.astype(np.float32)

        xs = np.zeros((c.NSH_P, c.IN_F), dtype=np.float32)
        xs[: c.NSH] = x[cc * c.NSH : (cc + 1) * c.NSH]
        xT = np.ascontiguousarray(xs.T).astype(NPBF)  # [IN_F, NSH_P]

        dinv_pc = np.ones(c.NSH_P, dtype=np.float32)
        dinv_pc[: c.NSH] = dinv_all[cc * c.NSH : (cc + 1) * c.NSH]
        dinvT = dinv_pc.reshape(c.T, P).T.copy()  # [P, T]

        brel = np.full(c.T * P, 60000.0, dtype=np.float32)
        brel[: c.NSH] = batch[cc * c.NSH : (cc + 1) * c.NSH] - g_base[cc]
        batchT = brel.reshape(c.T, P).T.astype(np.float32)

        goff0 = (g_base[cc] + np.arange(P)).astype(np.int32).reshape(P, 1)
        goff1 = goff0 + P
        per_core.append(
            dict(xT=xT, dinvT=dinvT, cinvT=cinvT, eidx=eidx,
                 dlh=dlh.astype(NPBF), batchT=batchT, goff0=goff0,
                 goff1=goff1, nreal=nreal)
        )

    iota_r = np.tile(np.arange(P, dtype=np.float32), (P, 1)).astype(NPBF)
    iota256 = np.tile(np.arange(2 * P, dtype=np.float32), (P, 1)).astype(NPBF)
    ident = np.eye(P, dtype=np.float32).astype(NPBF)
    consts = dict(iota_r=iota_r, iota256=iota256, ident=ident)
    meta = dict(Bmat=Bmat, GCH=GCH, ECOLS=max(ECOLS, 8), consts=consts,
                dl_col=dl_col, cells=cells, NG=NG, g_base=g_base)
    return meta, per_core


def build_program(cfg, meta, debug=False):
    c = cfg
    Bmat = meta["Bmat"]
    GCH = meta["GCH"]
    ECOLS = meta["ECOLS"]
    dl_col = meta["dl_col"]
    cells = meta["cells"]
    NG = meta["NG"]
    H = c.HID
    cell_seq = {tj: gi for gi, tj in enumerate(cells)}

    nc = bacc.Bacc(
        "TRN2", target_bir_lowering=False, debug=debug, num_devices=c.NC
    )

    # ---- I/O ----
    xT_in = nc.dram_tensor("xT", [c.IN_F, c.NSH_P], BF16, kind="ExternalInput")
    dinv_in = nc.dram_tensor("dinvT", [P, c.T], F32, kind="ExternalInput")
    cinv_in = nc.dram_tensor("cinvT", [P, c.GB], F32, kind="ExternalInput")
    W_td = nc.dram_tensor("W_td", [c.IN_F, H], F32, kind="ExternalInput")
    W_bu = nc.dram_tensor("W_bu", [c.IN_F, H], F32, kind="ExternalInput")
    b_td = nc.dram_tensor("b_td", [H], F32, kind="ExternalInput")
    b_bu = nc.dram_tensor("b_bu", [H], F32, kind="ExternalInput")
    fc_W = nc.dram_tensor("fc_W", [4 * H, c.OUT_F], F32, kind="ExternalInput")
    fc_b = nc.dram_tensor("fc_b", [c.OUT_F], F32, kind="ExternalInput")
    eidx = nc.dram_tensor("eidx", [P, ECOLS], I16, kind="ExternalInput")
    dlh = nc.dram_tensor("dlh", [P, max(GCH, 1)], BF16, kind="ExternalInput")
    batchT = nc.dram_tensor("batchT", [P, c.T], F32, kind="ExternalInput")
    goff0 = nc.dram_tensor("goff0", [P, 1], I32, kind="ExternalInput")
    goff1 = nc.dram_tensor("goff1", [P, 1], I32, kind="ExternalInput")
    nreal = nc.dram_tensor("nreal", [1, NG], I32, kind="ExternalInput")
    iota_r = nc.dram_tensor("iota_r", [P, P], BF16, kind="ExternalInput")
    iota256_in = nc.dram_tensor("iota256", [P, 2 * P], BF16, kind="ExternalInput")
    ident_in = nc.dram_tensor("ident", [P, P], BF16, kind="ExternalInput")
    out = nc.dram_tensor("out", [c.G, c.OUT_F], F32, kind="ExternalOutput")

    # ---- internal DRAM ----
    hn_local = nc.dram_tensor("hn_local", [c.NSH_P, c.FW], BF16)
    hn_q = [
        nc.dram_tensor(f"hn_q{k}", [c.NC * c.QROWS[k], c.FW], BF16,
                       addr_space="Shared")
        for k in range(c.BANKS)
    ]
    pwin = nc.dram_tensor("pwin", [4 * P, c.FEAT], F32)
    pall = nc.dram_tensor("pall", [c.NC * 4 * P, c.FEAT], F32,
                          addr_space="Shared")

    groups = [list(range(c.NC))]

    with tile.TileContext(nc) as tc:
        with (
            tc.tile_pool(name="const", bufs=1) as cp,
            tc.tile_pool(name="sb", bufs=3) as sp,
            tc.tile_pool(name="ohb", bufs=2) as op_,
            nc.gpsimd.register("nr0") as r0,
            nc.gpsimd.register("nr1") as r1,
        ):
            regs = [r0, r1]
            nc.gpsimd.load_library(mlp_lib)

            # ---- constants ----
            iota_sb = cp.tile([P, P], BF16)
            iota256_sb = cp.tile([P, 2 * P], BF16)
            ident_sb = cp.tile([P, P], BF16)
            nc.sync.dma_start(iota_sb[:], iota_r[:])
            nc.sync.dma_start(iota256_sb[:], iota256_in[:])
            nc.sync.dma_start(ident_sb[:], ident_in[:])
            ident32_sb = cp.tile([P, P], F32)
            nc.scalar.activation(ident32_sb[:], ident_sb[:], AF.Copy)

            wcat = cp.tile([P, c.FW], BF16)
            nc.gpsimd.dma_start(wcat[:, 0:H], W_td[:])
            nc.gpsimd.dma_start(wcat[:, H : 2 * H], W_bu[:])

            ones_bf = cp.tile([P, 1], BF16)
            nc.vector.memset(ones_bf[:], 1.0)

            bcat = cp.tile([1, c.FW], BF16)
            nc.gpsimd.dma_start(bcat[0:1, 0:H], b_td[None, :])
            nc.gpsimd.dma_start(bcat[0:1, H : 2 * H], b_bu[None, :])
            ones_row = cp.tile([1, P], BF16)
            nc.vector.memset(ones_row[:], 1.0)
            bias_sb = cp.tile([P, c.FW], F32)

            fw0 = cp.tile([P, c.OUT_F], F32)
            fw1 = cp.tile([P, c.OUT_F], F32)
            nc.sync.dma_start(fw0[:], fc_W[0:P, :])
            nc.sync.dma_start(fw1[:], fc_W[P : 2 * P, :])
            fcb = cp.tile([c.OUT_F, 1], F32)
            nc.sync.dma_start(fcb[:, 0:1], fc_b[:, None])

            eidx_sb = cp.tile([P, ECOLS], I16)
            nc.sync.dma_start(eidx_sb[:], eidx[:])
            dl_sb = cp.tile([P, max(GCH, 1)], BF16)
            nc.sync.dma_start(dl_sb[:], dlh[:])
            batch_sb = cp.tile([P, c.T], F32)
            nc.sync.dma_start(batch_sb[:], batchT[:])
            goff0_sb = cp.tile([P, 1], I32)
            goff1_sb = cp.tile([P, 1], I32)
            nc.sync.dma_start(goff0_sb[:], goff0[:])
            nc.sync.dma_start(goff1_sb[:], goff1[:])
            nreal_sb = cp.tile([1, NG], I32)
            nc.sync.dma_start(nreal_sb[:], nreal[:])

            dinv_sb = cp.tile([P, c.T], F32)
            nc.sync.dma_start(dinv_sb[:], dinv_in[:])
            cinv_sb = cp.tile([P, c.GB], F32)
            nc.sync.dma_start(cinv_sb[:], cinv_in[:])

            def onehot_big(t, tag):
                g0 = dl_col[(t, 0)]
                gt = sum(int(Bmat[t, j]) for j in range(c.BANKS))
                oh = op_.tile([P, gt * P], BF16, tag=tag)
                nc.vector.tensor_tensor(
                    out=oh[:].rearrange("p (g d) -> p g d", d=P),
                    in0=iota_sb[:].unsqueeze(1).broadcast_to([P, gt, P]),
                    in1=dl_sb[:, g0 : g0 + gt].to_broadcast([P, gt, P]),
                    op=ALU.is_equal,
                )
                return oh, g0, gt

            # ---- P1/P2 per bank stripe, AllGather each stripe when ready ----
            with tc.tile_pool(name="ps12", bufs=2, space="PSUM") as pp:
                bias_ps = pp.tile([P, c.FW], F32, space="PSUM", tag="bias")
                nc.tensor.matmul(
                    bias_ps[:], lhsT=ones_row[0:1, :], rhs=bcat[0:1, :],
                    start=True, stop=True,
                )
                nc.vector.tensor_copy(bias_sb[:], bias_ps[:])

                for k in range(c.BANKS):
                    for t in range(int(c.QSTART[k]), int(c.QSTART[k + 1])):
                        xT_sb = sp.tile([P, P], BF16, tag="xTs")
                        nc.sync.dma_start(
                            xT_sb[:], xT_in[:, t * P : (t + 1) * P]
                        )
                        h_ps = pp.tile([P, c.FW], F32, space="PSUM", tag="h")
                        nc.tensor.matmul(
                            h_ps[:], lhsT=xT_sb[:], rhs=wcat[:], start=True,
                            stop=True,
                        )
                        hn = sp.tile([P, c.FW], BF16, tag="hn")
                        nc.vector.tensor_scalar(
                            out=hn[:], in0=h_ps[:],
                            scalar1=dinv_sb[:, t : t + 1],
                            scalar2=None, op0=ALU.mult,
                        )
                        nc.sync.dma_start(
                            hn_local[t * P : (t + 1) * P, :], hn[:]
                        )

                    r_lo = int(c.QSTART[k]) * P
                    nc.gpsimd.collective_compute(
                        "AllGather",
                        ALU.bypass,
                        ins=[hn_local[r_lo : r_lo + c.QROWS[k], :]],
                        outs=[hn_q[k][:]],
                        replica_groups=groups,
                    )

            # ---- P4: gather + scatter-add + feat + pooling ----
            # Bank-round order within tile groups: gathers for bank k are
            # issued together so the Pool engine only ever waits for
            # AllGather k (never queues behind a later bank's AllGather).
            # Per-tile PSUM accumulators live across the 4 bank passes.
            GT = 6  # tiles per group: PSUM is bank-granular — 6 accs + 2 pool
            with (
                tc.tile_pool(name="gat", bufs=8) as gp,
                tc.tile_pool(name="psacc", bufs=1, space="PSUM") as pa,
            ):
                pool_ps0 = pa.tile([P, c.FEAT], F32, space="PSUM")
                pool_ps1 = pa.tile([P, c.FEAT], F32, space="PSUM")
                n_gather = 0
                accs = {}
                first_bank = {}
                for t in range(c.T):
                    banks = [j for j in range(c.BANKS) if Bmat[t, j]]
                    first_bank[t] = banks[0] if banks else -1
                for g0 in range(0, c.T, GT):
                    tiles = range(g0, min(g0 + GT, c.T))
                    for t in tiles:
                        accs[t] = pa.tile([P, c.FW], F32, space="PSUM",
                                          tag=f"acc{t - g0}",
                                          name=f"acc_g{g0}_{t - g0}")
                    for j in range(c.BANKS):
                        for t in tiles:
                            B = int(Bmat[t, j])
                            if B == 0:
                                continue
                            cb = dl_col[(t, j)]
                            gt_t = gp.tile([P, B * P], BF16, tag="gt")
                            if n_gather < 8:
                                # -1 slots leave SBUF untouched; scrub the
                                # first use of each pool slot so no NaN
                                # garbage reaches the matmul inputs
                                nc.vector.memset(gt_t[:], 0.0)
                            gi = cell_seq[(t, j)]
                            reg = regs[n_gather % 2]
                            nc.gpsimd.reg_load(reg, nreal_sb[0:1, gi : gi + 1])
                            nc.gpsimd.dma_gather(
                                gt_t[:].rearrange("p (b e) -> p b e", e=P),
                                hn_q[j][:],
                                eidx_sb[:, cb * 8 : cb * 8 + B * 8],
                                B * P,
                                reg,
                                c.FW,
                                single_packet=(B * P <= 1024),
                            )
                            n_gather += 1
                            oh = op_.tile([P, B * P], BF16, tag="ohb2")
                            nc.vector.tensor_tensor(
                                out=oh[:].rearrange("p (g d) -> p g d", d=P),
                                in0=iota_sb[:].unsqueeze(1).broadcast_to(
                                    [P, B, P]
                                ),
                                in1=dl_sb[:, cb : cb + B].to_broadcast(
                                    [P, B, P]
                                ),
                                op=ALU.is_equal,
                            )
                            for q in range(B):
                                nc.tensor.matmul(
                                    accs[t][:],
                                    lhsT=oh[:, q * P : (q + 1) * P],
                                    rhs=gt_t[:, q * P : (q + 1) * P],
                                    start=(j == first_bank[t] and q == 0),
                                    stop=False,
                                )
                    for t in tiles:
                        acc = accs[t]
                        hno = sp.tile([P, c.FW], BF16, tag="hno")
                        nc.sync.dma_start(
                            hno[:], hn_local[t * P : (t + 1) * P, :]
                        )
                        nc.tensor.matmul(
                            acc[:], lhsT=ident_sb[:], rhs=hno[:],
                            start=(first_bank[t] < 0), stop=True,
                        )

                        ot = sp.tile([P, c.FW], F32, tag="ot")
                        nc.scalar.activation(
                            ot[:], acc[:], AF.Copy, scale=dinv_sb[:, t : t + 1]
                        )
                        nc.vector.tensor_tensor(
                            out=ot[:], in0=ot[:], in1=bias_sb[:], op=ALU.add
                        )
                        feat = sp.tile([P, c.FEAT], BF16, tag="feat")
                        nc.scalar.activation(feat[:, 0:H], ot[:, 0:H], AF.Relu)
                        nc.scalar.copy(feat[:, H : 2 * H], ot[:, 0:H])
                        nc.scalar.activation(
                            feat[:, 2 * H : 3 * H], ot[:, H : 2 * H], AF.Relu
                        )
                        nc.scalar.copy(feat[:, 3 * H : 4 * H], ot[:, H : 2 * H])

                        ohg = sp.tile([P, 2 * P], BF16, tag="ohg")
                        nc.vector.tensor_scalar(
                            out=ohg[:], in0=iota256_sb[:],
                            scalar1=batch_sb[:, t : t + 1], scalar2=None,
                            op0=ALU.is_equal,
                        )
                        nc.tensor.matmul(
                            pool_ps0[:], lhsT=ohg[:, 0:P], rhs=feat[:],
                            start=(t == 0), stop=(t == c.T - 1),
                        )
                        nc.tensor.matmul(
                            pool_ps1[:], lhsT=ohg[:, P : 2 * P], rhs=feat[:],
                            start=(t == 0), stop=(t == c.T - 1),
                        )

                # ---- P5: write local pooled window (zero guard rows on
                # both sides so the combine can read aligned 128-row blocks)
                zt = sp.tile([P, c.FEAT], F32, tag="zt")
                nc.vector.memset(zt[:], 0.0)
                nc.sync.dma_start(pwin[0:P, :], zt[:])
                nc.sync.dma_start(pwin[3 * P : 4 * P, :], zt[:])
                pp0 = sp.tile([P, c.FEAT], F32, tag="pp0")
                nc.vector.tensor_copy(pp0[:], pool_ps0[:])
                nc.sync.dma_start(pwin[P : 2 * P, :], pp0[:])
                pp1 = sp.tile([P, c.FEAT], F32, tag="pp1")
                nc.vector.tensor_copy(pp1[:], pool_ps1[:])
                nc.sync.dma_start(pwin[2 * P : 3 * P, :], pp1[:])

            # ---- P6: AllGather pooled windows, combine statically ----
            nc.gpsimd.collective_compute(
                "AllGather",
                ALU.bypass,
                ins=[pwin[:]],
                outs=[pall[:]],
                replica_groups=groups,
            )

            # ---- P7: mean, FC, log_softmax (replicated) ----
            with tc.tile_pool(name="ps7", bufs=2, space="PSUM") as pp:
                g_base = meta["g_base"]
                for b in range(c.GB):
                    h_rows = min(P, c.G - b * P)
                    tt = sp.tile([P, c.FEAT], F32, tag="tt")
                    nc.vector.memset(tt[:], 0.0)
                    for cc in range(c.NC):
                        d = b * P - int(g_base[cc])
                        if d <= -P or d >= 2 * P:
                            continue  # no overlap with this core's window
                        stg = sp.tile([P, c.FEAT], F32, tag="stg")
                        r0 = cc * 4 * P + P + d  # guard rows make this valid
                        nc.sync.dma_start(stg[:], pall[r0 : r0 + P, :])
                        nc.vector.tensor_tensor(
                            out=tt[:], in0=tt[:], in1=stg[:], op=ALU.add
                        )
                    mean_sb = sp.tile([P, 4 * H], F32, tag="mean")
                    nc.vector.tensor_scalar(
                        out=mean_sb[:], in0=tt[:, 0 : 4 * H],
                        scalar1=cinv_sb[:, b : b + 1], scalar2=None,
                        op0=ALU.mult,
                    )
                    lg_ps = pp.tile([P, P], F32, space="PSUM", tag="lg")
                    for half in range(2):
                        tp_ps = pp.tile([P, P], F32, space="PSUM", tag="tp")
                        nc.tensor.transpose(
                            tp_ps[:], mean_sb[:, half * P : (half + 1) * P],
                            ident32_sb[:],
                        )
                        mt = sp.tile([P, P], F32, tag="mt")
                        nc.vector.tensor_copy(mt[:], tp_ps[:])
                        nc.tensor.matmul(
                            lg_ps[0 : c.OUT_F, :],
                            lhsT=(fw0 if half == 0 else fw1)[:],
                            rhs=mt[:],
                            start=(half == 0),
                            stop=(half == 1),
                        )
                    lgb = sp.tile([c.OUT_F, P], F32, tag="lgb")
                    nc.vector.tensor_scalar(
                        out=lgb[:], in0=lg_ps[0 : c.OUT_F, :],
                        scalar1=fcb[:, 0:1], scalar2=None, op0=ALU.add,
                    )
                    tr_ps = pp.tile([P, c.OUT_F], F32, space="PSUM", tag="tr")
                    nc.tensor.transpose(
                        tr_ps[:], lgb[:], ident32_sb[0 : c.OUT_F, 0 : c.OUT_F]
                    )
                    ls = sp.tile([P, c.OUT_F], F32, tag="ls")
                    nc.vector.tensor_copy(ls[:], tr_ps[:])
                    mx = sp.tile([P, 1], F32, tag="mx")
                    nc.vector.reduce_max(mx[:], ls[:], axis=mybir.AxisListType.X)
                    nc.vector.tensor_scalar(
                        out=ls[:], in0=ls[:], scalar1=mx[:, 0:1], scalar2=None,
                        op0=ALU.subtract,
                    )
                    ex = sp.tile([P, c.OUT_F], F32, tag="ex")
                    nc.scalar.activation(ex[:], ls[:], AF.Exp)
                    sm = sp.tile([P, 1], F32, tag="sm")
                    nc.vector.reduce_sum(sm[:], ex[:], axis=mybir.AxisListType.X)
                    nc.scalar.activation(sm[:], sm[:], AF.Ln)
                    nc.vector.tensor_scalar(
                        out=ls[:], in0=ls[:], scalar1=sm[:, 0:1], scalar2=None,
                        op0=ALU.subtract,
                    )
                    nc.sync.dma_start(
                        out[b * P : b * P + h_rows, :], ls[0:h_rows, :]
                    )

    nc.compile()
    return nc


def make_in_maps(cfg, meta, per_core, W_td, b_td, W_bu, b_bu, fc_W, fc_b):
    cst = meta["consts"]
    in_maps = []
    for cc in range(cfg.NC):
        pc = per_core[cc]
        in_maps.append(
            {
                "xT": pc["xT"],
                "dinvT": pc["dinvT"],
                "cinvT": pc["cinvT"],
                "W_td": np.asarray(W_td, dtype=np.float32),
                "W_bu": np.asarray(W_bu, dtype=np.float32),
                "b_td": np.asarray(b_td, dtype=np.float32),
                "b_bu": np.asarray(b_bu, dtype=np.float32),
                "fc_W": np.asarray(fc_W, dtype=np.float32),
                "fc_b": np.asarray(fc_b, dtype=np.float32),
                "eidx": pc["eidx"],
                "dlh": pc["dlh"],
                "batchT": pc["batchT"],
                "goff0": pc["goff0"],
                "goff1": pc["goff1"],
                "nreal": pc["nreal"],
                "iota_r": cst["iota_r"],
                "iota256": cst["iota256"],
                "ident": cst["ident"],
            }
        )
    return in_maps


def prep_and_build(cfg, inputs, debug=False):
    x = np.asarray(inputs["x"], dtype=np.float32)
    edge_index = np.asarray(inputs["edge_index"])
    batch = np.asarray(inputs["batch"]).astype(np.int64)
    meta, per_core = host_prep(cfg, x, edge_index, batch)
    nc = build_program(cfg, meta, debug=debug)
    in_maps = make_in_maps(
        cfg, meta, per_core,
        inputs["W_td"], inputs["b_td"], inputs["W_bu"], inputs["b_bu"],
        inputs["fc_W"], inputs["fc_b"],
    )
    return nc, in_maps


def run(cfg, inputs, debug=False, trace=False):
    nc, in_maps = prep_and_build(cfg, inputs, debug=debug)
    res = run_bass_kernel_spmd(nc, in_maps, list(range(cfg.NC)), trace=trace)
    out = res.results[0]["out"].astype(np.float32)
    return out, res


def full_cfg():
    return Cfg(
        n_nodes=100000, n_graphs=1000, n_cores=8, banks=4,
        in_f=128, hid_f=64, out_f=4,
    )


def kernel(**inputs):
    out, _ = run(full_cfg(), inputs)
    return out



# revision 23
# speedup vs baseline: 1.0050x; 1.0050x over previous
"""BiGCN (two fused GCNConv + graph mean-pool + FC + log_softmax) on 8 trn2 cores.

Strategy (graph/data parallel, partitioned by destination node range):
  - core c owns nodes [c*NSH, (c+1)*NSH) as edge destinations
  - host sorts edges into per-(dst-tile, table-bank) cells, padded to 128-slot
    chunks with -1 indices (skipped by the gather HW); per-core real counts
    are fed through gpsimd registers so pad slots cost no DMA descriptors
  - the bf16 Hn table is built as 4 bank stripes, each AllGathered separately
    so bank-0 gathers start while later banks are still being produced
  - device: degree count via one-hot matmul -> dinv = 1/sqrt(deg+1)
            Hn = (x @ [W_td | W_bu]) * dinv  (bf16), 4x AllGather -> banks
            dma_gather Hn[src] rows per cell, one-hot matmul scatter into PSUM
            out[d] = dinv[d]*(sum + Hn[d]) + b ; feat = [relu(td),td,relu(bu),bu]
            graph pooling via one-hot matmul, indirect-scatter + AllReduce,
            FC + log_softmax computed replicated on every core.
  - one-hot matrices are built in one DVE tensor_tensor per tile using
    broadcast access patterns (iota row vs per-slot dst-local values).
"""

import math

import numpy as np
import ml_dtypes

import concourse.bass as bass
import concourse.bacc as bacc
import concourse.mybir as mybir
import concourse.tile as tile
from concourse.bass import IndirectOffsetOnAxis
from concourse.bass_utils import run_bass_kernel_spmd
from concourse.library_config import mlp as mlp_lib

BF16 = mybir.dt.bfloat16
F32 = mybir.dt.float32
I16 = mybir.dt.int16
I32 = mybir.dt.int32
AF = mybir.ActivationFunctionType
ALU = mybir.AluOpType
NPBF = ml_dtypes.bfloat16

P = 128  # partitions / tile height


def _split_even(n, k):
    base = n // k
    rem = n % k
    return [base + (1 if i < rem else 0) for i in range(k)]


class Cfg:
    def __init__(self, n_nodes, n_graphs, n_cores, banks, in_f, hid_f, out_f):
        assert n_nodes % n_cores == 0
        self.N = n_nodes
        self.G = n_graphs
        self.NC = n_cores
        self.NSH = n_nodes // n_cores  # nodes per core
        self.T = math.ceil(self.NSH / P)  # dst tiles per core
        self.NSH_P = self.T * P  # padded shard rows
        self.BANKS = min(banks, self.T)
        # bank k holds the stripe of tiles [qt_start[k], qt_start[k+1]) from
        # every core: bank rows = NC * qrows[k]
        self.QT = _split_even(self.T, self.BANKS)  # tiles per bank stripe
        self.QSTART = np.concatenate([[0], np.cumsum(self.QT)])  # tile starts
        self.QROWS = [q * P for q in self.QT]
        for k in range(self.BANKS):
            assert self.NC * self.QROWS[k] <= 32767, "bank idx must fit int16"
        self.IN_F = in_f
        self.HID = hid_f
        self.FW = 2 * hid_f
        assert self.FW == P and in_f == P
        self.OUT_F = out_f
        self.FEAT = 4 * hid_f
        self.GB = math.ceil(self.G / P)
        self.PART_ROWS = (self.G + 2 * P + P - 1) // P * P


def host_prep(cfg, x, edge_index, batch):
    """Build per-core edge grids + constants. Returns (meta, per_core_inputs)."""
    c = cfg
    src = edge_index[0].astype(np.int64)
    dst = edge_index[1].astype(np.int64)
    assert src.min() >= 0 and src.max() < c.N and dst.min() >= 0 and dst.max() < c.N

    # host-side degree normalization (in-degree incl self-loop)
    deg = np.bincount(dst, minlength=c.N).astype(np.float64) + 1.0
    dinv_all = (1.0 / np.sqrt(deg)).astype(np.float32)

    # host-side graph-size reciprocals for mean pooling
    cnt = np.maximum(np.bincount(np.asarray(batch), minlength=c.G), 1)
    cinv = np.zeros(c.GB * P, dtype=np.float32)
    cinv[: c.G] = 1.0 / cnt.astype(np.float64)
    cinvT = cinv.reshape(c.GB, P).T.copy()  # [P, GB]

    qstart_rows = c.QSTART[:-1] * P  # local row where each bank stripe starts
    sc = src // c.NSH  # owner core of src
    so = src % c.NSH  # local row of src
    stile = so // P
    bank = np.searchsorted(c.QSTART[1:], stile, side="right")
    lidx = sc * np.asarray(c.QROWS)[bank] + (so - qstart_rows[bank])

    owner = dst // c.NSH
    tloc = (dst % c.NSH) // P
    dl = ((dst % c.NSH) % P).astype(np.int64)

    ncell = c.NC * c.T * c.BANKS
    cell = (owner * c.T + tloc) * c.BANKS + bank
    order = np.argsort(cell, kind="stable")
    cell_s = cell[order]
    lidx_s = lidx[order]
    dl_s = dl[order]
    counts = np.bincount(cell_s, minlength=ncell).reshape(c.NC, c.T, c.BANKS)
    starts = np.zeros(ncell + 1, dtype=np.int64)
    np.cumsum(counts.reshape(-1), out=starts[1:])

    Bmat = (-(-counts // P)).max(axis=0)  # [T, BANKS] chunks per cell
    GCH = int(Bmat.sum())
    ECOLS = GCH * 8

    g_base = np.empty(c.NC, dtype=np.int64)
    for cc in range(c.NC):
        b = batch[cc * c.NSH : (cc + 1) * c.NSH]
        g_base[cc] = int(b[0])
        assert int(b[-1]) - int(b[0]) < 2 * P, "graph span exceeds 2 blocks"

    # chunk columns ordered (t, j, q); shared by dl, eidx and the gather seq
    dl_col = {}
    col = 0
    cells = []  # active cells in order
    for t in range(c.T):
        for j in range(c.BANKS):
            dl_col[(t, j)] = col
            if Bmat[t, j]:
                cells.append((t, j))
            col += int(Bmat[t, j])
    NG = max(len(cells), 1)

    per_core = []
    for cc in range(c.NC):
        eidx = np.zeros((P, max(ECOLS, 8)), dtype=np.int16)
        dlh = np.full((P, max(GCH, 1)), 200.0, dtype=np.float32)
        nreal = np.zeros((1, NG), dtype=np.int32)
        for gi, (t, j) in enumerate(cells):
            B = int(Bmat[t, j])
            ci = (cc * c.T + t) * c.BANKS + j
            s0, s1 = starts[ci], starts[ci + 1]
            n = int(s1 - s0)
            slots = B * P
            li = np.full(slots, -1, dtype=np.int64)
            dv = np.full(slots, 200.0, dtype=np.float64)
            li[:n] = lidx_s[s0:s1]
            dv[:n] = dl_s[s0:s1]
            if n == 0:
                li[0] = 0  # keep >=1 real idx (sim/ucode edge case)
                n = 1
            nreal[0, gi] = n
            w = li.reshape(slots // 16, 16).T.astype(np.int16)
            cb = dl_col[(t, j)]
            eidx[:, cb * 8 : cb * 8 + B * 8] = np.tile(w, (8, 1))
            dlh[:, cb : cb + B] = dv.reshape(B, P).T.astype(np.float32)

        xs = np.zeros((c.NSH_P, c.IN_F), dtype=np.float32)
        xs[: c.NSH] = x[cc * c.NSH : (cc + 1) * c.NSH]
        xT = np.ascontiguousarray(xs.T).astype(NPBF)  # [IN_F, NSH_P]

        dinv_pc = np.ones(c.NSH_P, dtype=np.float32)
        dinv_pc[: c.NSH] = dinv_all[cc * c.NSH : (cc + 1) * c.NSH]
        dinvT = dinv_pc.reshape(c.T, P).T.copy()  # [P, T]

        brel = np.full(c.T * P, 60000.0, dtype=np.float32)
        brel[: c.NSH] = batch[cc * c.NSH : (cc + 1) * c.NSH] - g_base[cc]
        batchT = brel.reshape(c.T, P).T.astype(np.float32)

        goff0 = (g_base[cc] + np.arange(P)).astype(np.int32).reshape(P, 1)
        goff1 = goff0 + P
        per_core.append(
            dict(xT=xT, dinvT=dinvT, cinvT=cinvT, eidx=eidx,
                 dlh=dlh.astype(NPBF), batchT=batchT, goff0=goff0,
                 goff1=goff1, nreal=nreal)
        )

    iota_r = np.tile(np.arange(P, dtype=np.float32), (P, 1)).astype(NPBF)
    iota256 = np.tile(np.arange(2 * P, dtype=np.float32), (P, 1)).astype(NPBF)
    ident = np.eye(P, dtype=np.float32).astype(NPBF)
    consts = dict(iota_r=iota_r, iota256=iota256, ident=ident)
    meta = dict(Bmat=Bmat, GCH=GCH, ECOLS=max(ECOLS, 8), consts=consts,
                dl_col=dl_col, cells=cells, NG=NG, g_base=g_base)
    return meta, per_core


def build_program(cfg, meta, debug=False):
    c = cfg
    Bmat = meta["Bmat"]
    GCH = meta["GCH"]
    ECOLS = meta["ECOLS"]
    dl_col = meta["dl_col"]
    cells = meta["cells"]
    NG = meta["NG"]
    H = c.HID
    cell_seq = {tj: gi for gi, tj in enumerate(cells)}

    nc = bacc.Bacc(
        "TRN2", target_bir_lowering=False, debug=debug, num_devices=c.NC
    )

    # ---- I/O ----
    xT_in = nc.dram_tensor("xT", [c.IN_F, c.NSH_P], BF16, kind="ExternalInput")
    dinv_in = nc.dram_tensor("dinvT", [P, c.T], F32, kind="ExternalInput")
    cinv_in = nc.dram_tensor("cinvT", [P, c.GB], F32, kind="ExternalInput")
    W_td = nc.dram_tensor("W_td", [c.IN_F, H], F32, kind="ExternalInput")
    W_bu = nc.dram_tensor("W_bu", [c.IN_F, H], F32, kind="ExternalInput")
    b_td = nc.dram_tensor("b_td", [H], F32, kind="ExternalInput")
    b_bu = nc.dram_tensor("b_bu", [H], F32, kind="ExternalInput")
    fc_W = nc.dram_tensor("fc_W", [4 * H, c.OUT_F], F32, kind="ExternalInput")
    fc_b = nc.dram_tensor("fc_b", [c.OUT_F], F32, kind="ExternalInput")
    eidx = nc.dram_tensor("eidx", [P, ECOLS], I16, kind="ExternalInput")
    dlh = nc.dram_tensor("dlh", [P, max(GCH, 1)], BF16, kind="ExternalInput")
    batchT = nc.dram_tensor("batchT", [P, c.T], F32, kind="ExternalInput")
    goff0 = nc.dram_tensor("goff0", [P, 1], I32, kind="ExternalInput")
    goff1 = nc.dram_tensor("goff1", [P, 1], I32, kind="ExternalInput")
    nreal = nc.dram_tensor("nreal", [1, NG], I32, kind="ExternalInput")
    iota_r = nc.dram_tensor("iota_r", [P, P], BF16, kind="ExternalInput")
    iota256_in = nc.dram_tensor("iota256", [P, 2 * P], BF16, kind="ExternalInput")
    ident_in = nc.dram_tensor("ident", [P, P], BF16, kind="ExternalInput")
    out = nc.dram_tensor("out", [c.G, c.OUT_F], F32, kind="ExternalOutput")

    # ---- internal DRAM ----
    hn_local = nc.dram_tensor("hn_local", [c.NSH_P, c.FW], BF16)
    hn_q = [
        nc.dram_tensor(f"hn_q{k}", [c.NC * c.QROWS[k], c.FW], BF16,
                       addr_space="Shared")
        for k in range(c.BANKS)
    ]
    pwin = nc.dram_tensor("pwin", [4 * P, c.FEAT], BF16)
    pall = nc.dram_tensor("pall", [c.NC * 4 * P, c.FEAT], BF16,
                          addr_space="Shared")

    groups = [list(range(c.NC))]

    with tile.TileContext(nc) as tc:
        with (
            tc.tile_pool(name="const", bufs=1) as cp,
            tc.tile_pool(name="sb", bufs=3) as sp,
            tc.tile_pool(name="ohb", bufs=2) as op_,
            nc.gpsimd.register("nr0") as r0,
            nc.gpsimd.register("nr1") as r1,
        ):
            regs = [r0, r1]
            nc.gpsimd.load_library(mlp_lib)

            # ---- constants ----
            iota_sb = cp.tile([P, P], BF16)
            iota256_sb = cp.tile([P, 2 * P], BF16)
            ident_sb = cp.tile([P, P], BF16)
            nc.sync.dma_start(iota_sb[:], iota_r[:])
            nc.sync.dma_start(iota256_sb[:], iota256_in[:])
            nc.sync.dma_start(ident_sb[:], ident_in[:])
            ident32_sb = cp.tile([P, P], F32)
            nc.scalar.activation(ident32_sb[:], ident_sb[:], AF.Copy)

            wcat = cp.tile([P, c.FW], BF16)
            nc.gpsimd.dma_start(wcat[:, 0:H], W_td[:])
            nc.gpsimd.dma_start(wcat[:, H : 2 * H], W_bu[:])

            ones_bf = cp.tile([P, 1], BF16)
            nc.vector.memset(ones_bf[:], 1.0)

            bcat = cp.tile([1, c.FW], BF16)
            nc.gpsimd.dma_start(bcat[0:1, 0:H], b_td[None, :])
            nc.gpsimd.dma_start(bcat[0:1, H : 2 * H], b_bu[None, :])
            ones_row = cp.tile([1, P], BF16)
            nc.vector.memset(ones_row[:], 1.0)
            bias_sb = cp.tile([P, c.FW], F32)

            fw0 = cp.tile([P, c.OUT_F], F32)
            fw1 = cp.tile([P, c.OUT_F], F32)
            nc.sync.dma_start(fw0[:], fc_W[0:P, :])
            nc.sync.dma_start(fw1[:], fc_W[P : 2 * P, :])
            fcb = cp.tile([c.OUT_F, 1], F32)
            nc.sync.dma_start(fcb[:, 0:1], fc_b[:, None])

            eidx_sb = cp.tile([P, ECOLS], I16)
            dl_sb = cp.tile([P, max(GCH, 1)], BF16)
            batch_sb = cp.tile([P, c.T], F32)
            nc.sync.dma_start(batch_sb[:], batchT[:])
            goff0_sb = cp.tile([P, 1], I32)
            goff1_sb = cp.tile([P, 1], I32)
            nc.sync.dma_start(goff0_sb[:], goff0[:])
            nc.sync.dma_start(goff1_sb[:], goff1[:])
            nreal_sb = cp.tile([1, NG], I32)

            dinv_sb = cp.tile([P, c.T], F32)
            nc.sync.dma_start(dinv_sb[:], dinv_in[:])
            cinv_sb = cp.tile([P, c.GB], F32)
            nc.sync.dma_start(cinv_sb[:], cinv_in[:])

            def onehot_big(t, tag):
                g0 = dl_col[(t, 0)]
                gt = sum(int(Bmat[t, j]) for j in range(c.BANKS))
                oh = op_.tile([P, gt * P], BF16, tag=tag)
                nc.vector.tensor_tensor(
                    out=oh[:].rearrange("p (g d) -> p g d", d=P),
                    in0=iota_sb[:].unsqueeze(1).broadcast_to([P, gt, P]),
                    in1=dl_sb[:, g0 : g0 + gt].to_broadcast([P, gt, P]),
                    op=ALU.is_equal,
                )
                return oh, g0, gt

            # ---- P1/P2 per bank stripe, AllGather each stripe when ready ----
            with tc.tile_pool(name="ps12", bufs=2, space="PSUM") as pp:
                bias_ps = pp.tile([P, c.FW], F32, space="PSUM", tag="bias")
                nc.tensor.matmul(
                    bias_ps[:], lhsT=ones_row[0:1, :], rhs=bcat[0:1, :],
                    start=True, stop=True,
                )
                nc.vector.tensor_copy(bias_sb[:], bias_ps[:])

                for k in range(c.BANKS):
                    for t in range(int(c.QSTART[k]), int(c.QSTART[k + 1])):
                        xT_sb = sp.tile([P, P], BF16, tag="xTs")
                        nc.sync.dma_start(
                            xT_sb[:], xT_in[:, t * P : (t + 1) * P]
                        )
                        h_ps = pp.tile([P, c.FW], F32, space="PSUM", tag="h")
                        nc.tensor.matmul(
                            h_ps[:], lhsT=xT_sb[:], rhs=wcat[:], start=True,
                            stop=True,
                        )
                        hn = sp.tile([P, c.FW], BF16, tag="hn")
                        nc.vector.tensor_scalar(
                            out=hn[:], in0=h_ps[:],
                            scalar1=dinv_sb[:, t : t + 1],
                            scalar2=None, op0=ALU.mult,
                        )
                        nc.sync.dma_start(
                            hn_local[t * P : (t + 1) * P, :], hn[:]
                        )

                    r_lo = int(c.QSTART[k]) * P
                    nc.gpsimd.collective_compute(
                        "AllGather",
                        ALU.bypass,
                        ins=[hn_local[r_lo : r_lo + c.QROWS[k], :]],
                        outs=[hn_q[k][:]],
                        replica_groups=groups,
                    )

            # edge tables load behind the P1 DMAs/AllGathers on purpose:
            # nothing needs them before the first gather call
            nc.sync.dma_start(eidx_sb[:], eidx[:])
            nc.sync.dma_start(dl_sb[:], dlh[:])
            nc.sync.dma_start(nreal_sb[:], nreal[:])

            # ---- P4: gather + scatter-add + feat + pooling ----
            # Bank-round order within tile groups: gathers for bank k are
            # issued together so the Pool engine only ever waits for
            # AllGather k (never queues behind a later bank's AllGather).
            # Per-tile PSUM accumulators live across the 4 bank passes.
            GT = 6  # tiles per group: PSUM is bank-granular — 6 accs + 2 pool
            with (
                tc.tile_pool(name="gat", bufs=8) as gp,
                tc.tile_pool(name="psacc", bufs=1, space="PSUM") as pa,
            ):
                pool_ps0 = pa.tile([P, c.FEAT], F32, space="PSUM")
                pool_ps1 = pa.tile([P, c.FEAT], F32, space="PSUM")
                n_gather = 0
                accs = {}
                first_bank = {}
                for t in range(c.T):
                    banks = [j for j in range(c.BANKS) if Bmat[t, j]]
                    first_bank[t] = banks[0] if banks else -1
                for g0 in range(0, c.T, GT):
                    tiles = range(g0, min(g0 + GT, c.T))
                    for t in tiles:
                        accs[t] = pa.tile([P, c.FW], F32, space="PSUM",
                                          tag=f"acc{t - g0}",
                                          name=f"acc_g{g0}_{t - g0}")
                    for j in range(c.BANKS):
                        for t in tiles:
                            B = int(Bmat[t, j])
                            if B == 0:
                                continue
                            cb = dl_col[(t, j)]
                            gt_t = gp.tile([P, B * P], BF16, tag="gt")
                            if n_gather < 8:
                                # -1 slots leave SBUF untouched; scrub the
                                # first use of each pool slot so no NaN
                                # garbage reaches the matmul inputs
                                nc.vector.memset(gt_t[:], 0.0)
                            gi = cell_seq[(t, j)]
                            reg = regs[n_gather % 2]
                            nc.gpsimd.reg_load(reg, nreal_sb[0:1, gi : gi + 1])
                            nc.gpsimd.dma_gather(
                                gt_t[:].rearrange("p (b e) -> p b e", e=P),
                                hn_q[j][:],
                                eidx_sb[:, cb * 8 : cb * 8 + B * 8],
                                B * P,
                                reg,
                                c.FW,
                                single_packet=(B * P <= 1024),
                            )
                            n_gather += 1
                            oh = op_.tile([P, B * P], BF16, tag="ohb2")
                            nc.vector.tensor_tensor(
                                out=oh[:].rearrange("p (g d) -> p g d", d=P),
                                in0=iota_sb[:].unsqueeze(1).broadcast_to(
                                    [P, B, P]
                                ),
                                in1=dl_sb[:, cb : cb + B].to_broadcast(
                                    [P, B, P]
                                ),
                                op=ALU.is_equal,
                            )
                            for q in range(B):
                                nc.tensor.matmul(
                                    accs[t][:],
                                    lhsT=oh[:, q * P : (q + 1) * P],
                                    rhs=gt_t[:, q * P : (q + 1) * P],
                                    start=(j == first_bank[t] and q == 0),
                                    stop=False,
                                )
                    for t in tiles:
                        acc = accs[t]
                        hno = sp.tile([P, c.FW], BF16, tag="hno")
                        nc.sync.dma_start(
                            hno[:], hn_local[t * P : (t + 1) * P, :]
                        )
                        nc.tensor.matmul(
                            acc[:], lhsT=ident_sb[:], rhs=hno[:],
                            start=(first_bank[t] < 0), stop=True,
                        )

                        ot = sp.tile([P, c.FW], F32, tag="ot")
                        nc.scalar.activation(
                            ot[:], acc[:], AF.Copy, scale=dinv_sb[:, t : t + 1]
                        )
                        nc.vector.tensor_tensor(
                            out=ot[:], in0=ot[:], in1=bias_sb[:], op=ALU.add
                        )
                        feat = sp.tile([P, c.FEAT], BF16, tag="feat")
                        nc.scalar.activation(feat[:, 0:H], ot[:, 0:H], AF.Relu)
                        nc.scalar.copy(feat[:, H : 2 * H], ot[:, 0:H])
                        nc.scalar.activation(
                            feat[:, 2 * H : 3 * H], ot[:, H : 2 * H], AF.Relu
                        )
                        nc.scalar.copy(feat[:, 3 * H : 4 * H], ot[:, H : 2 * H])

                        ohg = sp.tile([P, 2 * P], BF16, tag="ohg")
                        nc.vector.tensor_scalar(
                            out=ohg[:], in0=iota256_sb[:],
                            scalar1=batch_sb[:, t : t + 1], scalar2=None,
                            op0=ALU.is_equal,
                        )
                        nc.tensor.matmul(
                            pool_ps0[:], lhsT=ohg[:, 0:P], rhs=feat[:],
                            start=(t == 0), stop=(t == c.T - 1),
                        )
                        nc.tensor.matmul(
                            pool_ps1[:], lhsT=ohg[:, P : 2 * P], rhs=feat[:],
                            start=(t == 0), stop=(t == c.T - 1),
                        )

                # ---- P5: write local pooled window (zero guard rows on
                # both sides so the combine can read aligned 128-row blocks)
                zt = sp.tile([P, c.FEAT], BF16, tag="zt")
                nc.vector.memset(zt[:], 0.0)
                nc.sync.dma_start(pwin[0:P, :], zt[:])
                nc.sync.dma_start(pwin[3 * P : 4 * P, :], zt[:])
                pp0 = sp.tile([P, c.FEAT], BF16, tag="pp0")
                nc.vector.tensor_copy(pp0[:], pool_ps0[:])
                nc.sync.dma_start(pwin[P : 2 * P, :], pp0[:])
                pp1 = sp.tile([P, c.FEAT], BF16, tag="pp1")
                nc.vector.tensor_copy(pp1[:], pool_ps1[:])
                nc.sync.dma_start(pwin[2 * P : 3 * P, :], pp1[:])

            # ---- P6: AllGather pooled windows, combine statically ----
            nc.gpsimd.collective_compute(
                "AllGather",
                ALU.bypass,
                ins=[pwin[:]],
                outs=[pall[:]],
                replica_groups=groups,
            )

            # ---- P7: mean, FC, log_softmax (replicated) ----
            with tc.tile_pool(name="ps7", bufs=2, space="PSUM") as pp:
                g_base = meta["g_base"]
                for b in range(c.GB):
                    h_rows = min(P, c.G - b * P)
                    tt = sp.tile([P, c.FEAT], F32, tag="tt")
                    nc.vector.memset(tt[:], 0.0)
                    for cc in range(c.NC):
                        d = b * P - int(g_base[cc])
                        if d <= -P or d >= 2 * P:
                            continue  # no overlap with this core's window
                        stg = sp.tile([P, c.FEAT], BF16, tag="stg")
                        r0 = cc * 4 * P + P + d  # guard rows make this valid
                        nc.sync.dma_start(stg[:], pall[r0 : r0 + P, :])
                        nc.vector.tensor_tensor(
                            out=tt[:], in0=tt[:], in1=stg[:], op=ALU.add
                        )
                    mean_sb = sp.tile([P, 4 * H], F32, tag="mean")
                    nc.vector.tensor_scalar(
                        out=mean_sb[:], in0=tt[:, 0 : 4 * H],
                        scalar1=cinv_sb[:, b : b + 1], scalar2=None,
                        op0=ALU.mult,
                    )
                    lg_ps = pp.tile([P, P], F32, space="PSUM", tag="lg")
                    for half in range(2):
                        tp_ps = pp.tile([P, P], F32, space="PSUM", tag="tp")
                        nc.tensor.transpose(
                            tp_ps[:], mean_sb[:, half * P : (half + 1) * P],
                            ident32_sb[:],
                        )
                        mt = sp.tile([P, P], F32, tag="mt")
                        nc.vector.tensor_copy(mt[:], tp_ps[:])
                        nc.tensor.matmul(
                            lg_ps[0 : c.OUT_F, :],
                            lhsT=(fw0 if half == 0 else fw1)[:],
                            rhs=mt[:],
                            start=(half == 0),
                            stop=(half == 1),
                        )
                    lgb = sp.tile([c.OUT_F, P], F32, tag="lgb")
                    nc.vector.tensor_scalar(
                        out=lgb[:], in0=lg_ps[0 : c.OUT_F, :],
                        scalar1=fcb[:, 0:1], scalar2=None, op0=ALU.add,
                    )
                    tr_ps = pp.tile([P, c.OUT_F], F32, space="PSUM", tag="tr")
                    nc.tensor.transpose(
                        tr_ps[:], lgb[:], ident32_sb[0 : c.OUT_F, 0 : c.OUT_F]
                    )
                    ls = sp.tile([P, c.OUT_F], F32, tag="ls")
                    nc.vector.tensor_copy(ls[:], tr_ps[:])
                    mx = sp.tile([P, 1], F32, tag="mx")
                    nc.vector.reduce_max(mx[:], ls[:], axis=mybir.AxisListType.X)
                    nc.vector.tensor_scalar(
                        out=ls[:], in0=ls[:], scalar1=mx[:, 0:1], scalar2=None,
                        op0=ALU.subtract,
                    )
                    ex = sp.tile([P, c.OUT_F], F32, tag="ex")
                    nc.scalar.activation(ex[:], ls[:], AF.Exp)
                    sm = sp.tile([P, 1], F32, tag="sm")
                    nc.vector.reduce_sum(sm[:], ex[:], axis=mybir.AxisListType.X)
                    nc.scalar.activation(sm[:], sm[:], AF.Ln)
                    nc.vector.tensor_scalar(
                        out=ls[:], in0=ls[:], scalar1=sm[:, 0:1], scalar2=None,
                        op0=ALU.subtract,
                    )
                    nc.sync.dma_start(
                        out[b * P : b * P + h_rows, :], ls[0:h_rows, :]
                    )

    nc.compile()
    return nc


def make_in_maps(cfg, meta, per_core, W_td, b_td, W_bu, b_bu, fc_W, fc_b):
    cst = meta["consts"]
    in_maps = []
    for cc in range(cfg.NC):
        pc = per_core[cc]
        in_maps.append(
            {
                "xT": pc["xT"],
                "dinvT": pc["dinvT"],
                "cinvT": pc["cinvT"],
                "W_td": np.asarray(W_td, dtype=np.float32),
                "W_bu": np.asarray(W_bu, dtype=np.float32),
                "b_td": np.asarray(b_td, dtype=np.float32),
                "b_bu": np.asarray(b_bu, dtype=np.float32),
                "fc_W": np.asarray(fc_W, dtype=np.float32),
                "fc_b": np.asarray(fc_b, dtype=np.float32),
                "eidx": pc["eidx"],
                "dlh": pc["dlh"],
                "batchT": pc["batchT"],
                "goff0": pc["goff0"],
                "goff1": pc["goff1"],
                "nreal": pc["nreal"],
                "iota_r": cst["iota_r"],
                "iota256": cst["iota256"],
                "ident": cst["ident"],
            }
        )
    return in_maps


def prep_and_build(cfg, inputs, debug=False):
    x = np.asarray(inputs["x"], dtype=np.float32)
    edge_index = np.asarray(inputs["edge_index"])
    batch = np.asarray(inputs["batch"]).astype(np.int64)
    meta, per_core = host_prep(cfg, x, edge_index, batch)
    nc = build_program(cfg, meta, debug=debug)
    in_maps = make_in_maps(
        cfg, meta, per_core,
        inputs["W_td"], inputs["b_td"], inputs["W_bu"], inputs["b_bu"],
        inputs["fc_W"], inputs["fc_b"],
    )
    return nc, in_maps


def run(cfg, inputs, debug=False, trace=False):
    nc, in_maps = prep_and_build(cfg, inputs, debug=debug)
    res = run_bass_kernel_spmd(nc, in_maps, list(range(cfg.NC)), trace=trace)
    out = res.results[0]["out"].astype(np.float32)
    return out, res


def full_cfg():
    return Cfg(
        n_nodes=100000, n_graphs=1000, n_cores=8, banks=4,
        in_f=128, hid_f=64, out_f=4,
    )


def kernel(**inputs):
    out, _ = run(full_cfg(), inputs)
    return out



# revision 24
# speedup vs baseline: 1.0084x; 1.0034x over previous
"""BiGCN (two fused GCNConv + graph mean-pool + FC + log_softmax) on 8 trn2 cores.

Strategy (graph/data parallel, partitioned by destination node range):
  - core c owns nodes [c*NSH, (c+1)*NSH) as edge destinations
  - host sorts edges into per-(dst-tile, table-bank) cells, padded to 128-slot
    chunks with -1 indices (skipped by the gather HW); per-core real counts
    are fed through gpsimd registers so pad slots cost no DMA descriptors
  - the bf16 Hn table is built as 4 bank stripes, each AllGathered separately
    so bank-0 gathers start while later banks are still being produced
  - device: degree count via one-hot matmul -> dinv = 1/sqrt(deg+1)
            Hn = (x @ [W_td | W_bu]) * dinv  (bf16), 4x AllGather -> banks
            dma_gather Hn[src] rows per cell, one-hot matmul scatter into PSUM
            out[d] = dinv[d]*(sum + Hn[d]) + b ; feat = [relu(td),td,relu(bu),bu]
            graph pooling via one-hot matmul, indirect-scatter + AllReduce,
            FC + log_softmax computed replicated on every core.
  - one-hot matrices are built in one DVE tensor_tensor per tile using
    broadcast access patterns (iota row vs per-slot dst-local values).
"""

import math

import numpy as np
import ml_dtypes

import concourse.bass as bass
import concourse.bacc as bacc
import concourse.mybir as mybir
import concourse.tile as tile
from concourse.bass import IndirectOffsetOnAxis
from concourse.bass_utils import run_bass_kernel_spmd
from concourse.library_config import mlp as mlp_lib

BF16 = mybir.dt.bfloat16
F32 = mybir.dt.float32
I16 = mybir.dt.int16
I32 = mybir.dt.int32
AF = mybir.ActivationFunctionType
ALU = mybir.AluOpType
NPBF = ml_dtypes.bfloat16

P = 128  # partitions / tile height


def _split_even(n, k):
    base = n // k
    rem = n % k
    return [base + (1 if i < rem else 0) for i in range(k)]


class Cfg:
    def __init__(self, n_nodes, n_graphs, n_cores, banks, in_f, hid_f, out_f):
        assert n_nodes % n_cores == 0
        self.N = n_nodes
        self.G = n_graphs
        self.NC = n_cores
        self.NSH = n_nodes // n_cores  # nodes per core
        self.T = math.ceil(self.NSH / P)  # dst tiles per core
        self.NSH_P = self.T * P  # padded shard rows
        self.BANKS = min(banks, self.T)
        # bank k holds the stripe of tiles [qt_start[k], qt_start[k+1]) from
        # every core: bank rows = NC * qrows[k]
        self.QT = _split_even(self.T, self.BANKS)  # tiles per bank stripe
        self.QSTART = np.concatenate([[0], np.cumsum(self.QT)])  # tile starts
        self.QROWS = [q * P for q in self.QT]
        for k in range(self.BANKS):
            assert self.NC * self.QROWS[k] <= 32767, "bank idx must fit int16"
        self.IN_F = in_f
        self.HID = hid_f
        self.FW = 2 * hid_f
        assert self.FW == P and in_f == P
        self.OUT_F = out_f
        self.FEAT = 4 * hid_f
        self.GB = math.ceil(self.G / P)
        self.PART_ROWS = (self.G + 2 * P + P - 1) // P * P


def host_prep(cfg, x, edge_index, batch):
    """Build per-core edge grids + constants. Returns (meta, per_core_inputs)."""
    c = cfg
    src = edge_index[0].astype(np.int64)
    dst = edge_index[1].astype(np.int64)
    assert src.min() >= 0 and src.max() < c.N and dst.min() >= 0 and dst.max() < c.N

    # host-side degree normalization (in-degree incl self-loop)
    deg = np.bincount(dst, minlength=c.N).astype(np.float64) + 1.0
    dinv_all = (1.0 / np.sqrt(deg)).astype(np.float32)

    # host-side graph-size reciprocals for mean pooling
    cnt = np.maximum(np.bincount(np.asarray(batch), minlength=c.G), 1)
    cinv = np.zeros(c.GB * P, dtype=np.float32)
    cinv[: c.G] = 1.0 / cnt.astype(np.float64)
    cinvT = cinv.reshape(c.GB, P).T.copy()  # [P, GB]

    qstart_rows = c.QSTART[:-1] * P  # local row where each bank stripe starts
    sc = src // c.NSH  # owner core of src
    so = src % c.NSH  # local row of src
    stile = so // P
    bank = np.searchsorted(c.QSTART[1:], stile, side="right")
    lidx = sc * np.asarray(c.QROWS)[bank] + (so - qstart_rows[bank])

    owner = dst // c.NSH
    tloc = (dst % c.NSH) // P
    dl = ((dst % c.NSH) % P).astype(np.int64)

    ncell = c.NC * c.T * c.BANKS
    cell = (owner * c.T + tloc) * c.BANKS + bank
    order = np.argsort(cell, kind="stable")
    cell_s = cell[order]
    lidx_s = lidx[order]
    dl_s = dl[order]
    counts = np.bincount(cell_s, minlength=ncell).reshape(c.NC, c.T, c.BANKS)
    starts = np.zeros(ncell + 1, dtype=np.int64)
    np.cumsum(counts.reshape(-1), out=starts[1:])

    Bmat = (-(-counts // P)).max(axis=0)  # [T, BANKS] chunks per cell
    GCH = int(Bmat.sum())
    ECOLS = GCH * 8

    g_base = np.empty(c.NC, dtype=np.int64)
    for cc in range(c.NC):
        b = batch[cc * c.NSH : (cc + 1) * c.NSH]
        g_base[cc] = int(b[0])
        assert int(b[-1]) - int(b[0]) < 2 * P, "graph span exceeds 2 blocks"

    # chunk columns ordered (t, j, q); shared by dl, eidx and the gather seq
    dl_col = {}
    col = 0
    cells = []  # active cells in order
    for t in range(c.T):
        for j in range(c.BANKS):
            dl_col[(t, j)] = col
            if Bmat[t, j]:
                cells.append((t, j))
            col += int(Bmat[t, j])
    NG = max(len(cells), 1)

    per_core = []
    for cc in range(c.NC):
        eidx = np.zeros((P, max(ECOLS, 8)), dtype=np.int16)
        dlh = np.full((P, max(GCH, 1)), 200.0, dtype=np.float32)
        nreal = np.zeros((1, NG), dtype=np.int32)
        for gi, (t, j) in enumerate(cells):
            B = int(Bmat[t, j])
            ci = (cc * c.T + t) * c.BANKS + j
            s0, s1 = starts[ci], starts[ci + 1]
            n = int(s1 - s0)
            slots = B * P
            li = np.full(slots, -1, dtype=np.int64)
            dv = np.full(slots, 200.0, dtype=np.float64)
            li[:n] = lidx_s[s0:s1]
            dv[:n] = dl_s[s0:s1]
            if n == 0:
                li[0] = 0  # keep >=1 real idx (sim/ucode edge case)
                n = 1
            nreal[0, gi] = n
            w = li.reshape(slots // 16, 16).T.astype(np.int16)
            cb = dl_col[(t, j)]
            eidx[:, cb * 8 : cb * 8 + B * 8] = np.tile(w, (8, 1))
            dlh[:, cb : cb + B] = dv.reshape(B, P).T.astype(np.float32)

        xs = np.zeros((c.NSH_P, c.IN_F), dtype=np.float32)
        xs[: c.NSH] = x[cc * c.NSH : (cc + 1) * c.NSH]
        xT = np.ascontiguousarray(xs.T).astype(NPBF)  # [IN_F, NSH_P]

        dinv_pc = np.ones(c.NSH_P, dtype=np.float32)
        dinv_pc[: c.NSH] = dinv_all[cc * c.NSH : (cc + 1) * c.NSH]
        dinvT = dinv_pc.reshape(c.T, P).T.copy()  # [P, T]

        brel = np.full(c.T * P, 60000.0, dtype=np.float32)
        brel[: c.NSH] = batch[cc * c.NSH : (cc + 1) * c.NSH] - g_base[cc]
        batchT = brel.reshape(c.T, P).T.astype(np.float32)

        goff0 = (g_base[cc] + np.arange(P)).astype(np.int32).reshape(P, 1)
        goff1 = goff0 + P
        per_core.append(
            dict(xT=xT, dinvT=dinvT, cinvT=cinvT, eidx=eidx,
                 dlh=dlh.astype(NPBF), batchT=batchT, goff0=goff0,
                 goff1=goff1, nreal=nreal)
        )

    iota_r = np.tile(np.arange(P, dtype=np.float32), (P, 1)).astype(NPBF)
    iota256 = np.tile(np.arange(2 * P, dtype=np.float32), (P, 1)).astype(NPBF)
    ident = np.eye(P, dtype=np.float32).astype(NPBF)
    consts = dict(iota_r=iota_r, iota256=iota256, ident=ident)
    meta = dict(Bmat=Bmat, GCH=GCH, ECOLS=max(ECOLS, 8), consts=consts,
                dl_col=dl_col, cells=cells, NG=NG, g_base=g_base)
    return meta, per_core


def build_program(cfg, meta, debug=False):
    c = cfg
    Bmat = meta["Bmat"]
    GCH = meta["GCH"]
    ECOLS = meta["ECOLS"]
    dl_col = meta["dl_col"]
    cells = meta["cells"]
    NG = meta["NG"]
    H = c.HID
    cell_seq = {tj: gi for gi, tj in enumerate(cells)}

    nc = bacc.Bacc(
        "TRN2", target_bir_lowering=False, debug=debug, num_devices=c.NC
    )

    # ---- I/O ----
    xT_in = nc.dram_tensor("xT", [c.IN_F, c.NSH_P], BF16, kind="ExternalInput")
    dinv_in = nc.dram_tensor("dinvT", [P, c.T], F32, kind="ExternalInput")
    cinv_in = nc.dram_tensor("cinvT", [P, c.GB], F32, kind="ExternalInput")
    W_td = nc.dram_tensor("W_td", [c.IN_F, H], F32, kind="ExternalInput")
    W_bu = nc.dram_tensor("W_bu", [c.IN_F, H], F32, kind="ExternalInput")
    b_td = nc.dram_tensor("b_td", [H], F32, kind="ExternalInput")
    b_bu = nc.dram_tensor("b_bu", [H], F32, kind="ExternalInput")
    fc_W = nc.dram_tensor("fc_W", [4 * H, c.OUT_F], F32, kind="ExternalInput")
    fc_b = nc.dram_tensor("fc_b", [c.OUT_F], F32, kind="ExternalInput")
    eidx = nc.dram_tensor("eidx", [P, ECOLS], I16, kind="ExternalInput")
    dlh = nc.dram_tensor("dlh", [P, max(GCH, 1)], BF16, kind="ExternalInput")
    batchT = nc.dram_tensor("batchT", [P, c.T], F32, kind="ExternalInput")
    goff0 = nc.dram_tensor("goff0", [P, 1], I32, kind="ExternalInput")
    goff1 = nc.dram_tensor("goff1", [P, 1], I32, kind="ExternalInput")
    nreal = nc.dram_tensor("nreal", [1, NG], I32, kind="ExternalInput")
    iota_r = nc.dram_tensor("iota_r", [P, P], BF16, kind="ExternalInput")
    iota256_in = nc.dram_tensor("iota256", [P, 2 * P], BF16, kind="ExternalInput")
    ident_in = nc.dram_tensor("ident", [P, P], BF16, kind="ExternalInput")
    out = nc.dram_tensor("out", [c.G, c.OUT_F], F32, kind="ExternalOutput")

    # ---- internal DRAM ----
    hn_local = nc.dram_tensor("hn_local", [c.NSH_P, c.FW], BF16)
    hn_q = [
        nc.dram_tensor(f"hn_q{k}", [c.NC * c.QROWS[k], c.FW], BF16,
                       addr_space="Shared")
        for k in range(c.BANKS)
    ]
    pwin = nc.dram_tensor("pwin", [4 * P, c.FEAT], BF16)
    pall = nc.dram_tensor("pall", [c.NC * 4 * P, c.FEAT], BF16,
                          addr_space="Shared")

    groups = [list(range(c.NC))]

    with tile.TileContext(nc) as tc:
        with (
            tc.tile_pool(name="const", bufs=1) as cp,
            tc.tile_pool(name="sb", bufs=3) as sp,
            tc.tile_pool(name="ohb", bufs=2) as op_,
            nc.gpsimd.register("nr0") as r0,
            nc.gpsimd.register("nr1") as r1,
        ):
            regs = [r0, r1]
            nc.gpsimd.load_library(mlp_lib)

            # ---- constants ----
            iota_sb = cp.tile([P, P], BF16)
            iota256_sb = cp.tile([P, 2 * P], BF16)
            ident_sb = cp.tile([P, P], BF16)
            nc.sync.dma_start(iota_sb[:], iota_r[:])
            nc.sync.dma_start(iota256_sb[:], iota256_in[:])
            nc.sync.dma_start(ident_sb[:], ident_in[:])
            ident32_sb = cp.tile([P, P], F32)
            nc.scalar.activation(ident32_sb[:], ident_sb[:], AF.Copy)

            wcat = cp.tile([P, c.FW], BF16)
            nc.gpsimd.dma_start(wcat[:, 0:H], W_td[:])
            nc.gpsimd.dma_start(wcat[:, H : 2 * H], W_bu[:])

            ones_bf = cp.tile([P, 1], BF16)
            nc.vector.memset(ones_bf[:], 1.0)

            bcat = cp.tile([1, c.FW], BF16)
            nc.gpsimd.dma_start(bcat[0:1, 0:H], b_td[None, :])
            nc.gpsimd.dma_start(bcat[0:1, H : 2 * H], b_bu[None, :])
            ones_row = cp.tile([1, P], BF16)
            nc.vector.memset(ones_row[:], 1.0)
            bias_sb = cp.tile([P, c.FW], F32)

            fw0 = cp.tile([P, c.OUT_F], F32)
            fw1 = cp.tile([P, c.OUT_F], F32)
            nc.sync.dma_start(fw0[:], fc_W[0:P, :])
            nc.sync.dma_start(fw1[:], fc_W[P : 2 * P, :])
            fcb = cp.tile([c.OUT_F, 1], F32)
            nc.sync.dma_start(fcb[:, 0:1], fc_b[:, None])

            eidx_sb = cp.tile([P, ECOLS], I16)
            nc.sync.dma_start(eidx_sb[:], eidx[:])
            dl_sb = cp.tile([P, max(GCH, 1)], BF16)
            nc.sync.dma_start(dl_sb[:], dlh[:])
            batch_sb = cp.tile([P, c.T], F32)
            nc.sync.dma_start(batch_sb[:], batchT[:])
            goff0_sb = cp.tile([P, 1], I32)
            goff1_sb = cp.tile([P, 1], I32)
            nc.sync.dma_start(goff0_sb[:], goff0[:])
            nc.sync.dma_start(goff1_sb[:], goff1[:])
            nreal_sb = cp.tile([1, NG], I32)
            nc.sync.dma_start(nreal_sb[:], nreal[:])

            dinv_sb = cp.tile([P, c.T], F32)
            nc.sync.dma_start(dinv_sb[:], dinv_in[:])
            cinv_sb = cp.tile([P, c.GB], F32)
            nc.sync.dma_start(cinv_sb[:], cinv_in[:])

            def onehot_big(t, tag):
                g0 = dl_col[(t, 0)]
                gt = sum(int(Bmat[t, j]) for j in range(c.BANKS))
                oh = op_.tile([P, gt * P], BF16, tag=tag)
                nc.vector.tensor_tensor(
                    out=oh[:].rearrange("p (g d) -> p g d", d=P),
                    in0=iota_sb[:].unsqueeze(1).broadcast_to([P, gt, P]),
                    in1=dl_sb[:, g0 : g0 + gt].to_broadcast([P, gt, P]),
                    op=ALU.is_equal,
                )
                return oh, g0, gt

            # ---- P1/P2 per bank stripe, AllGather each stripe when ready ----
            with tc.tile_pool(name="ps12", bufs=2, space="PSUM") as pp:
                bias_ps = pp.tile([P, c.FW], F32, space="PSUM", tag="bias")
                nc.tensor.matmul(
                    bias_ps[:], lhsT=ones_row[0:1, :], rhs=bcat[0:1, :],
                    start=True, stop=True,
                )
                nc.vector.tensor_copy(bias_sb[:], bias_ps[:])

                for k in range(c.BANKS):
                    for t in range(int(c.QSTART[k]), int(c.QSTART[k + 1])):
                        xT_sb = sp.tile([P, P], BF16, tag="xTs")
                        nc.sync.dma_start(
                            xT_sb[:], xT_in[:, t * P : (t + 1) * P]
                        )
                        h_ps = pp.tile([P, c.FW], F32, space="PSUM", tag="h")
                        nc.tensor.matmul(
                            h_ps[:], lhsT=xT_sb[:], rhs=wcat[:], start=True,
                            stop=True,
                        )
                        hn = sp.tile([P, c.FW], BF16, tag="hn")
                        nc.vector.tensor_scalar(
                            out=hn[:], in0=h_ps[:],
                            scalar1=dinv_sb[:, t : t + 1],
                            scalar2=None, op0=ALU.mult,
                        )
                        nc.sync.dma_start(
                            hn_local[t * P : (t + 1) * P, :], hn[:]
                        )

                    r_lo = int(c.QSTART[k]) * P
                    nc.gpsimd.collective_compute(
                        "AllGather",
                        ALU.bypass,
                        ins=[hn_local[r_lo : r_lo + c.QROWS[k], :]],
                        outs=[hn_q[k][:]],
                        replica_groups=groups,
                    )

            # ---- P4: gather + scatter-add + feat + pooling ----
            # Bank-round order within tile groups: gathers for bank k are
            # issued together so the Pool engine only ever waits for
            # AllGather k (never queues behind a later bank's AllGather).
            # Per-tile PSUM accumulators live across the 4 bank passes.
            GT = 6  # tiles per group: PSUM is bank-granular — 6 accs + 2 pool
            with (
                tc.tile_pool(name="gat", bufs=8) as gp,
                tc.tile_pool(name="psacc", bufs=1, space="PSUM") as pa,
            ):
                pool_ps0 = pa.tile([P, c.FEAT], F32, space="PSUM")
                pool_ps1 = pa.tile([P, c.FEAT], F32, space="PSUM")
                n_gather = 0
                accs = {}
                first_bank = {}
                for t in range(c.T):
                    banks = [j for j in range(c.BANKS) if Bmat[t, j]]
                    first_bank[t] = banks[0] if banks else -1
                for g0 in range(0, c.T, GT):
                    tiles = range(g0, min(g0 + GT, c.T))
                    for t in tiles:
                        accs[t] = pa.tile([P, c.FW], F32, space="PSUM",
                                          tag=f"acc{t - g0}",
                                          name=f"acc_g{g0}_{t - g0}")
                    for j in range(c.BANKS):
                        for t in tiles:
                            B = int(Bmat[t, j])
                            if B == 0:
                                continue
                            cb = dl_col[(t, j)]
                            gt_t = gp.tile([P, B * P], BF16, tag="gt")
                            if n_gather < 8:
                                # -1 slots leave SBUF untouched; scrub the
                                # first use of each pool slot so no NaN
                                # garbage reaches the matmul inputs
                                nc.vector.memset(gt_t[:], 0.0)
                            gi = cell_seq[(t, j)]
                            reg = regs[n_gather % 2]
                            nc.gpsimd.reg_load(reg, nreal_sb[0:1, gi : gi + 1])
                            nc.gpsimd.dma_gather(
                                gt_t[:].rearrange("p (b e) -> p b e", e=P),
                                hn_q[j][:],
                                eidx_sb[:, cb * 8 : cb * 8 + B * 8],
                                B * P,
                                reg,
                                c.FW,
                                single_packet=(B * P <= 1024),
                            )
                            n_gather += 1
                            oh = op_.tile([P, B * P], BF16, tag="ohb2")
                            nc.vector.tensor_tensor(
                                out=oh[:].rearrange("p (g d) -> p g d", d=P),
                                in0=iota_sb[:].unsqueeze(1).broadcast_to(
                                    [P, B, P]
                                ),
                                in1=dl_sb[:, cb : cb + B].to_broadcast(
                                    [P, B, P]
                                ),
                                op=ALU.is_equal,
                            )
                            for q in range(B):
                                nc.tensor.matmul(
                                    accs[t][:],
                                    lhsT=oh[:, q * P : (q + 1) * P],
                                    rhs=gt_t[:, q * P : (q + 1) * P],
                                    start=(j == first_bank[t] and q == 0),
                                    stop=False,
                                )
                    for t in tiles:
                        acc = accs[t]
                        hno = sp.tile([P, c.FW], BF16, tag="hno")
                        nc.sync.dma_start(
                            hno[:], hn_local[t * P : (t + 1) * P, :]
                        )
                        nc.tensor.matmul(
                            acc[:], lhsT=ident_sb[:], rhs=hno[:],
                            start=(first_bank[t] < 0), stop=True,
                        )

                        ot = sp.tile([P, c.FW], F32, tag="ot")
                        nc.scalar.activation(
                            ot[:], acc[:], AF.Copy, scale=dinv_sb[:, t : t + 1]
                        )
                        nc.vector.tensor_tensor(
                            out=ot[:], in0=ot[:], in1=bias_sb[:], op=ALU.add
                        )
                        feat = sp.tile([P, c.FEAT], BF16, tag="feat")
                        nc.scalar.activation(feat[:, 0:H], ot[:, 0:H], AF.Relu)
                        nc.scalar.copy(feat[:, H : 2 * H], ot[:, 0:H])
                        nc.scalar.activation(
                            feat[:, 2 * H : 3 * H], ot[:, H : 2 * H], AF.Relu
                        )
                        nc.scalar.copy(feat[:, 3 * H : 4 * H], ot[:, H : 2 * H])

                        ohg = sp.tile([P, 2 * P], BF16, tag="ohg")
                        nc.vector.tensor_scalar(
                            out=ohg[:], in0=iota256_sb[:],
                            scalar1=batch_sb[:, t : t + 1], scalar2=None,
                            op0=ALU.is_equal,
                        )
                        nc.tensor.matmul(
                            pool_ps0[:], lhsT=ohg[:, 0:P], rhs=feat[:],
                            start=(t == 0), stop=(t == c.T - 1),
                        )
                        nc.tensor.matmul(
                            pool_ps1[:], lhsT=ohg[:, P : 2 * P], rhs=feat[:],
                            start=(t == 0), stop=(t == c.T - 1),
                        )

                # ---- P5: write local pooled window (zero guard rows on
                # both sides so the combine can read aligned 128-row blocks)
                zt = sp.tile([P, c.FEAT], BF16, tag="zt")
                nc.vector.memset(zt[:], 0.0)
                nc.sync.dma_start(pwin[0:P, :], zt[:])
                nc.sync.dma_start(pwin[3 * P : 4 * P, :], zt[:])
                pp0 = sp.tile([P, c.FEAT], BF16, tag="pp0")
                nc.vector.tensor_copy(pp0[:], pool_ps0[:])
                nc.sync.dma_start(pwin[P : 2 * P, :], pp0[:])
                pp1 = sp.tile([P, c.FEAT], BF16, tag="pp1")
                nc.vector.tensor_copy(pp1[:], pool_ps1[:])
                nc.sync.dma_start(pwin[2 * P : 3 * P, :], pp1[:])

            # ---- P6: AllGather pooled windows, combine statically ----
            nc.gpsimd.collective_compute(
                "AllGather",
                ALU.bypass,
                ins=[pwin[:]],
                outs=[pall[:]],
                replica_groups=groups,
            )

            # ---- P7: mean, FC, log_softmax (replicated) ----
            with tc.tile_pool(name="ps7", bufs=2, space="PSUM") as pp:
                g_base = meta["g_base"]
                for b in range(c.GB):
                    h_rows = min(P, c.G - b * P)
                    tt = sp.tile([P, c.FEAT], F32, tag="tt")
                    nc.vector.memset(tt[:], 0.0)
                    for cc in range(c.NC):
                        d = b * P - int(g_base[cc])
                        if d <= -P or d >= 2 * P:
                            continue  # no overlap with this core's window
                        stg = sp.tile([P, c.FEAT], BF16, tag="stg")
                        r0 = cc * 4 * P + P + d  # guard rows make this valid
                        nc.sync.dma_start(stg[:], pall[r0 : r0 + P, :])
                        nc.vector.tensor_tensor(
                            out=tt[:], in0=tt[:], in1=stg[:], op=ALU.add
                        )
                    mean_sb = sp.tile([P, 4 * H], F32, tag="mean")
                    nc.vector.tensor_scalar(
                        out=mean_sb[:], in0=tt[:, 0 : 4 * H],
                        scalar1=cinv_sb[:, b : b + 1], scalar2=None,
                        op0=ALU.mult,
                    )
                    lg_ps = pp.tile([P, P], F32, space="PSUM", tag="lg")
                    for half in range(2):
                        tp_ps = pp.tile([P, P], F32, space="PSUM", tag="tp")
                        nc.tensor.transpose(
                            tp_ps[:], mean_sb[:, half * P : (half + 1) * P],
                            ident32_sb[:],
                        )
                        mt = sp.tile([P, P], F32, tag="mt")
                        nc.vector.tensor_copy(mt[:], tp_ps[:])
                        nc.tensor.matmul(
                            lg_ps[0 : c.OUT_F, :],
                            lhsT=(fw0 if half == 0 else fw1)[:],
                            rhs=mt[:],
                            start=(half == 0),
                            stop=(half == 1),
                        )
                    lgb = sp.tile([c.OUT_F, P], F32, tag="lgb")
                    nc.vector.tensor_scalar(
                        out=lgb[:], in0=lg_ps[0 : c.OUT_F, :],
                        scalar1=fcb[:, 0:1], scalar2=None, op0=ALU.add,
                    )
                    tr_ps = pp.tile([P, c.OUT_F], F32, space="PSUM", tag="tr")
                    nc.tensor.transpose(
                        tr_ps[:], lgb[:], ident32_sb[0 : c.OUT_F, 0 : c.OUT_F]
                    )
                    ls = sp.tile([P, c.OUT_F], F32, tag="ls")
                    nc.vector.tensor_copy(ls[:], tr_ps[:])
                    mx = sp.tile([P, 1], F32, tag="mx")
                    nc.vector.reduce_max(mx[:], ls[:], axis=mybir.AxisListType.X)
                    nc.vector.tensor_scalar(
                        out=ls[:], in0=ls[:], scalar1=mx[:, 0:1], scalar2=None,
                        op0=ALU.subtract,
                    )
                    ex = sp.tile([P, c.OUT_F], F32, tag="ex")
                    nc.scalar.activation(ex[:], ls[:], AF.Exp)
                    sm = sp.tile([P, 1], F32, tag="sm")
                    nc.vector.reduce_sum(sm[:], ex[:], axis=mybir.AxisListType.X)
                    nc.scalar.activation(sm[:], sm[:], AF.Ln)
                    nc.vector.tensor_scalar(
                        out=ls[:], in0=ls[:], scalar1=sm[:, 0:1], scalar2=None,
                        op0=ALU.subtract,
                    )
                    nc.sync.dma_start(
                        out[b * P : b * P + h_rows, :], ls[0:h_rows, :]
                    )

    nc.compile()
    return nc


def make_in_maps(cfg, meta, per_core, W_td, b_td, W_bu, b_bu, fc_W, fc_b):
    cst = meta["consts"]
    in_maps = []
    for cc in range(cfg.NC):
        pc = per_core[cc]
        in_maps.append(
            {
                "xT": pc["xT"],
                "dinvT": pc["dinvT"],
                "cinvT": pc["cinvT"],
                "W_td": np.asarray(W_td, dtype=np.float32),
                "W_bu": np.asarray(W_bu, dtype=np.float32),
                "b_td": np.asarray(b_td, dtype=np.float32),
                "b_bu": np.asarray(b_bu, dtype=np.float32),
                "fc_W": np.asarray(fc_W, dtype=np.float32),
                "fc_b": np.asarray(fc_b, dtype=np.float32),
                "eidx": pc["eidx"],
                "dlh": pc["dlh"],
                "batchT": pc["batchT"],
                "goff0": pc["goff0"],
                "goff1": pc["goff1"],
                "nreal": pc["nreal"],
                "iota_r": cst["iota_r"],
                "iota256": cst["iota256"],
                "ident": cst["ident"],
            }
        )
    return in_maps


def prep_and_build(cfg, inputs, debug=False):
    x = np.asarray(inputs["x"], dtype=np.float32)
    edge_index = np.asarray(inputs["edge_index"])
    batch = np.asarray(inputs["batch"]).astype(np.int64)
    meta, per_core = host_prep(cfg, x, edge_index, batch)
    nc = build_program(cfg, meta, debug=debug)
    in_maps = make_in_maps(
        cfg, meta, per_core,
        inputs["W_td"], inputs["b_td"], inputs["W_bu"], inputs["b_bu"],
        inputs["fc_W"], inputs["fc_b"],
    )
    return nc, in_maps


def run(cfg, inputs, debug=False, trace=False):
    nc, in_maps = prep_and_build(cfg, inputs, debug=debug)
    res = run_bass_kernel_spmd(nc, in_maps, list(range(cfg.NC)), trace=trace)
    out = res.results[0]["out"].astype(np.float32)
    return out, res


def full_cfg():
    return Cfg(
        n_nodes=100000, n_graphs=1000, n_cores=8, banks=4,
        in_f=128, hid_f=64, out_f=4,
    )


def kernel(**inputs):
    out, _ = run(full_cfg(), inputs)
    return out



# revision 26
# speedup vs baseline: 2.8137x; 2.7901x over previous
"""BiGCN (two fused GCNConv + graph mean-pool + FC + log_softmax) on 8 trn2 cores.

Strategy (graph/data parallel, partitioned by destination node range):
  - core c owns nodes [c*NSH, (c+1)*NSH) as edge destinations
  - host sorts edges into per-(dst-tile, table-bank) cells, padded to 128-slot
    chunks with -1 indices (skipped by the gather HW); per-core real counts
    are fed through gpsimd registers so pad slots cost no DMA descriptors
  - the bf16 Hn table is built as 4 bank stripes, each AllGathered separately
    so bank-0 gathers start while later banks are still being produced
  - host precomputes dinv = 1/sqrt(deg+1), graph-size reciprocals, and the
    bf16-transposed x shard, so the device does no degree work
  - device: Hn = (xT.T @ [W_td | W_bu]) * dinv  (bf16), 4x AllGather -> banks
            dma_gather Hn[src] rows per cell in BANK-ROUND order over tile
            groups (6 PSUM accumulators live across the 4 bank passes) so the
            Pool engine only ever waits on the bank it is gathering from;
            one-hot matmul scatter into the group's PSUM accs
            out[d] = dinv[d]*(sum + Hn[d]) + b ; feat = [relu(td),td,relu(bu),bu]
            graph pooling via one-hot matmul into guarded bf16 windows,
            one small AllGather + static window combine, then
            FC + log_softmax computed replicated on every core.
  - one-hot matrices are built in one DVE tensor_tensor per cell using
    broadcast access patterns (iota row vs per-slot dst-local values).
"""

import math

import numpy as np
import ml_dtypes

import concourse.bass as bass
import concourse.bacc as bacc
import concourse.mybir as mybir
import concourse.tile as tile
from concourse.bass import IndirectOffsetOnAxis
from concourse.bass_utils import run_bass_kernel_spmd
from concourse.library_config import mlp as mlp_lib

BF16 = mybir.dt.bfloat16
F32 = mybir.dt.float32
I16 = mybir.dt.int16
I32 = mybir.dt.int32
AF = mybir.ActivationFunctionType
ALU = mybir.AluOpType
NPBF = ml_dtypes.bfloat16

P = 128  # partitions / tile height


def _split_even(n, k):
    base = n // k
    rem = n % k
    return [base + (1 if i < rem else 0) for i in range(k)]


class Cfg:
    def __init__(self, n_nodes, n_graphs, n_cores, banks, in_f, hid_f, out_f):
        assert n_nodes % n_cores == 0
        self.N = n_nodes
        self.G = n_graphs
        self.NC = n_cores
        self.NSH = n_nodes // n_cores  # nodes per core
        self.T = math.ceil(self.NSH / P)  # dst tiles per core
        self.NSH_P = self.T * P  # padded shard rows
        self.BANKS = min(banks, self.T)
        # bank k holds the stripe of tiles [qt_start[k], qt_start[k+1]) from
        # every core: bank rows = NC * qrows[k]
        self.QT = _split_even(self.T, self.BANKS)  # tiles per bank stripe
        self.QSTART = np.concatenate([[0], np.cumsum(self.QT)])  # tile starts
        self.QROWS = [q * P for q in self.QT]
        for k in range(self.BANKS):
            assert self.NC * self.QROWS[k] <= 32767, "bank idx must fit int16"
        self.IN_F = in_f
        self.HID = hid_f
        self.FW = 2 * hid_f
        assert self.FW == P and in_f == P
        self.OUT_F = out_f
        self.FEAT = 4 * hid_f
        self.GB = math.ceil(self.G / P)
        self.PART_ROWS = (self.G + 2 * P + P - 1) // P * P


def host_prep(cfg, x, edge_index, batch):
    """Build per-core edge grids + constants. Returns (meta, per_core_inputs)."""
    c = cfg
    src = edge_index[0].astype(np.int64)
    dst = edge_index[1].astype(np.int64)
    assert src.min() >= 0 and src.max() < c.N and dst.min() >= 0 and dst.max() < c.N

    # host-side degree normalization (in-degree incl self-loop)
    deg = np.bincount(dst, minlength=c.N).astype(np.float64) + 1.0
    dinv_all = (1.0 / np.sqrt(deg)).astype(np.float32)

    # host-side graph-size reciprocals for mean pooling
    cnt = np.maximum(np.bincount(np.asarray(batch), minlength=c.G), 1)
    cinv = np.zeros(c.GB * P, dtype=np.float32)
    cinv[: c.G] = 1.0 / cnt.astype(np.float64)
    cinvT = cinv.reshape(c.GB, P).T.copy()  # [P, GB]

    qstart_rows = c.QSTART[:-1] * P  # local row where each bank stripe starts
    sc = src // c.NSH  # owner core of src
    so = src % c.NSH  # local row of src
    stile = so // P
    bank = np.searchsorted(c.QSTART[1:], stile, side="right")
    lidx = sc * np.asarray(c.QROWS)[bank] + (so - qstart_rows[bank])

    owner = dst // c.NSH
    tloc = (dst % c.NSH) // P
    dl = ((dst % c.NSH) % P).astype(np.int64)

    ncell = c.NC * c.T * c.BANKS
    cell = (owner * c.T + tloc) * c.BANKS + bank
    order = np.argsort(cell, kind="stable")
    cell_s = cell[order]
    lidx_s = lidx[order]
    dl_s = dl[order]
    counts = np.bincount(cell_s, minlength=ncell).reshape(c.NC, c.T, c.BANKS)
    starts = np.zeros(ncell + 1, dtype=np.int64)
    np.cumsum(counts.reshape(-1), out=starts[1:])

    Bmat = (-(-counts // P)).max(axis=0)  # [T, BANKS] chunks per cell
    GCH = int(Bmat.sum())
    ECOLS = GCH * 8

    g_base = np.empty(c.NC, dtype=np.int64)
    for cc in range(c.NC):
        b = batch[cc * c.NSH : (cc + 1) * c.NSH]
        g_base[cc] = int(b[0])
        assert int(b[-1]) - int(b[0]) < 2 * P, "graph span exceeds 2 blocks"

    # chunk columns ordered (t, j, q); shared by dl, eidx and the gather seq
    dl_col = {}
    col = 0
    cells = []  # active cells in order
    for t in range(c.T):
        for j in range(c.BANKS):
            dl_col[(t, j)] = col
            if Bmat[t, j]:
                cells.append((t, j))
            col += int(Bmat[t, j])
    NG = max(len(cells), 1)

    per_core = []
    for cc in range(c.NC):
        eidx = np.zeros((P, max(ECOLS, 8)), dtype=np.int16)
        dlh = np.full((P, max(GCH, 1)), 200.0, dtype=np.float32)
        nreal = np.zeros((1, NG), dtype=np.int32)
        for gi, (t, j) in enumerate(cells):
            B = int(Bmat[t, j])
            ci = (cc * c.T + t) * c.BANKS + j
            s0, s1 = starts[ci], starts[ci + 1]
            n = int(s1 - s0)
            slots = B * P
            li = np.full(slots, -1, dtype=np.int64)
            dv = np.full(slots, 200.0, dtype=np.float64)
            li[:n] = lidx_s[s0:s1]
            dv[:n] = dl_s[s0:s1]
            if n == 0:
                li[0] = 0  # keep >=1 real idx (sim/ucode edge case)
                n = 1
            nreal[0, gi] = n
            w = li.reshape(slots // 16, 16).T.astype(np.int16)
            cb = dl_col[(t, j)]
            eidx[:, cb * 8 : cb * 8 + B * 8] = np.tile(w, (8, 1))
            dlh[:, cb : cb + B] = dv.reshape(B, P).T.astype(np.float32)

        xs = np.zeros((c.NSH_P, c.IN_F), dtype=np.float32)
        xs[: c.NSH] = x[cc * c.NSH : (cc + 1) * c.NSH]
        xT = np.ascontiguousarray(xs.T).astype(NPBF)  # [IN_F, NSH_P]

        dinv_pc = np.ones(c.NSH_P, dtype=np.float32)
        dinv_pc[: c.NSH] = dinv_all[cc * c.NSH : (cc + 1) * c.NSH]
        dinvT = dinv_pc.reshape(c.T, P).T.copy()  # [P, T]

        brel = np.full(c.T * P, 60000.0, dtype=np.float32)
        brel[: c.NSH] = batch[cc * c.NSH : (cc + 1) * c.NSH] - g_base[cc]
        batchT = brel.reshape(c.T, P).T.astype(np.float32)

        goff0 = (g_base[cc] + np.arange(P)).astype(np.int32).reshape(P, 1)
        goff1 = goff0 + P
        per_core.append(
            dict(xT=xT, dinvT=dinvT, cinvT=cinvT, eidx=eidx,
                 dlh=dlh.astype(NPBF), batchT=batchT, goff0=goff0,
                 goff1=goff1, nreal=nreal)
        )

    iota_r = np.tile(np.arange(P, dtype=np.float32), (P, 1)).astype(NPBF)
    iota256 = np.tile(np.arange(2 * P, dtype=np.float32), (P, 1)).astype(NPBF)
    ident = np.eye(P, dtype=np.float32).astype(NPBF)
    consts = dict(iota_r=iota_r, iota256=iota256, ident=ident)
    meta = dict(Bmat=Bmat, GCH=GCH, ECOLS=max(ECOLS, 8), consts=consts,
                dl_col=dl_col, cells=cells, NG=NG, g_base=g_base)
    return meta, per_core


def build_program(cfg, meta, debug=False):
    c = cfg
    Bmat = meta["Bmat"]
    GCH = meta["GCH"]
    ECOLS = meta["ECOLS"]
    dl_col = meta["dl_col"]
    cells = meta["cells"]
    NG = meta["NG"]
    H = c.HID
    cell_seq = {tj: gi for gi, tj in enumerate(cells)}

    nc = bacc.Bacc(
        "TRN2", target_bir_lowering=False, debug=debug, num_devices=c.NC,
        num_swdge_queues=4,
    )

    # ---- I/O ----
    xT_in = nc.dram_tensor("xT", [c.IN_F, c.NSH_P], BF16, kind="ExternalInput")
    dinv_in = nc.dram_tensor("dinvT", [P, c.T], F32, kind="ExternalInput")
    cinv_in = nc.dram_tensor("cinvT", [P, c.GB], F32, kind="ExternalInput")
    W_td = nc.dram_tensor("W_td", [c.IN_F, H], F32, kind="ExternalInput")
    W_bu = nc.dram_tensor("W_bu", [c.IN_F, H], F32, kind="ExternalInput")
    b_td = nc.dram_tensor("b_td", [H], F32, kind="ExternalInput")
    b_bu = nc.dram_tensor("b_bu", [H], F32, kind="ExternalInput")
    fc_W = nc.dram_tensor("fc_W", [4 * H, c.OUT_F], F32, kind="ExternalInput")
    fc_b = nc.dram_tensor("fc_b", [c.OUT_F], F32, kind="ExternalInput")
    eidx = nc.dram_tensor("eidx", [P, ECOLS], I16, kind="ExternalInput")
    dlh = nc.dram_tensor("dlh", [P, max(GCH, 1)], BF16, kind="ExternalInput")
    batchT = nc.dram_tensor("batchT", [P, c.T], F32, kind="ExternalInput")
    goff0 = nc.dram_tensor("goff0", [P, 1], I32, kind="ExternalInput")
    goff1 = nc.dram_tensor("goff1", [P, 1], I32, kind="ExternalInput")
    nreal = nc.dram_tensor("nreal", [1, NG], I32, kind="ExternalInput")
    iota_r = nc.dram_tensor("iota_r", [P, P], BF16, kind="ExternalInput")
    iota256_in = nc.dram_tensor("iota256", [P, 2 * P], BF16, kind="ExternalInput")
    ident_in = nc.dram_tensor("ident", [P, P], BF16, kind="ExternalInput")
    out = nc.dram_tensor("out", [c.G, c.OUT_F], F32, kind="ExternalOutput")

    # ---- internal DRAM ----
    hn_local = nc.dram_tensor("hn_local", [c.NSH_P, c.FW], BF16)
    hn_q = [
        nc.dram_tensor(f"hn_q{k}", [c.NC * c.QROWS[k], c.FW], BF16,
                       addr_space="Shared")
        for k in range(c.BANKS)
    ]
    pwin = nc.dram_tensor("pwin", [4 * P, c.FEAT], BF16)
    pall = nc.dram_tensor("pall", [c.NC * 4 * P, c.FEAT], BF16,
                          addr_space="Shared")

    groups = [list(range(c.NC))]

    with tile.TileContext(nc) as tc:
        with (
            tc.tile_pool(name="const", bufs=1) as cp,
            tc.tile_pool(name="sb", bufs=3) as sp,
            tc.tile_pool(name="ohb", bufs=2) as op_,
            nc.gpsimd.register("nr0") as r0,
            nc.gpsimd.register("nr1") as r1,
        ):
            regs = [r0, r1]
            nc.gpsimd.load_library(mlp_lib)

            # ---- constants ----
            iota_sb = cp.tile([P, P], BF16)
            iota256_sb = cp.tile([P, 2 * P], BF16)
            ident_sb = cp.tile([P, P], BF16)
            nc.sync.dma_start(iota_sb[:], iota_r[:])
            nc.sync.dma_start(iota256_sb[:], iota256_in[:])
            nc.sync.dma_start(ident_sb[:], ident_in[:])
            ident32_sb = cp.tile([P, P], F32)
            nc.scalar.activation(ident32_sb[:], ident_sb[:], AF.Copy)

            wcat = cp.tile([P, c.FW], BF16)
            nc.gpsimd.dma_start(wcat[:, 0:H], W_td[:])
            nc.gpsimd.dma_start(wcat[:, H : 2 * H], W_bu[:])

            ones_bf = cp.tile([P, 1], BF16)
            nc.vector.memset(ones_bf[:], 1.0)

            bcat = cp.tile([1, c.FW], BF16)
            nc.gpsimd.dma_start(bcat[0:1, 0:H], b_td[None, :])
            nc.gpsimd.dma_start(bcat[0:1, H : 2 * H], b_bu[None, :])
            ones_row = cp.tile([1, P], BF16)
            nc.vector.memset(ones_row[:], 1.0)
            bias_sb = cp.tile([P, c.FW], F32)

            fw0 = cp.tile([P, c.OUT_F], F32)
            fw1 = cp.tile([P, c.OUT_F], F32)
            nc.sync.dma_start(fw0[:], fc_W[0:P, :])
            nc.sync.dma_start(fw1[:], fc_W[P : 2 * P, :])
            fcb = cp.tile([c.OUT_F, 1], F32)
            nc.sync.dma_start(fcb[:, 0:1], fc_b[:, None])

            eidx_sb = cp.tile([P, ECOLS], I16)
            nc.sync.dma_start(eidx_sb[:], eidx[:])
            dl_sb = cp.tile([P, max(GCH, 1)], BF16)
            nc.sync.dma_start(dl_sb[:], dlh[:])
            batch_sb = cp.tile([P, c.T], F32)
            nc.sync.dma_start(batch_sb[:], batchT[:])
            goff0_sb = cp.tile([P, 1], I32)
            goff1_sb = cp.tile([P, 1], I32)
            nc.sync.dma_start(goff0_sb[:], goff0[:])
            nc.sync.dma_start(goff1_sb[:], goff1[:])
            nreal_sb = cp.tile([1, NG], I32)
            nc.sync.dma_start(nreal_sb[:], nreal[:])

            dinv_sb = cp.tile([P, c.T], F32)
            nc.sync.dma_start(dinv_sb[:], dinv_in[:])
            cinv_sb = cp.tile([P, c.GB], F32)
            nc.sync.dma_start(cinv_sb[:], cinv_in[:])

            def onehot_big(t, tag):
                g0 = dl_col[(t, 0)]
                gt = sum(int(Bmat[t, j]) for j in range(c.BANKS))
                oh = op_.tile([P, gt * P], BF16, tag=tag)
                nc.vector.tensor_tensor(
                    out=oh[:].rearrange("p (g d) -> p g d", d=P),
                    in0=iota_sb[:].unsqueeze(1).broadcast_to([P, gt, P]),
                    in1=dl_sb[:, g0 : g0 + gt].to_broadcast([P, gt, P]),
                    op=ALU.is_equal,
                )
                return oh, g0, gt

            # ---- P1/P2 per bank stripe, AllGather each stripe when ready ----
            with tc.tile_pool(name="ps12", bufs=2, space="PSUM") as pp:
                bias_ps = pp.tile([P, c.FW], F32, space="PSUM", tag="bias")
                nc.tensor.matmul(
                    bias_ps[:], lhsT=ones_row[0:1, :], rhs=bcat[0:1, :],
                    start=True, stop=True,
                )
                nc.vector.tensor_copy(bias_sb[:], bias_ps[:])

                for k in range(c.BANKS):
                    for t in range(int(c.QSTART[k]), int(c.QSTART[k + 1])):
                        xT_sb = sp.tile([P, P], BF16, tag="xTs")
                        nc.sync.dma_start(
                            xT_sb[:], xT_in[:, t * P : (t + 1) * P]
                        )
                        h_ps = pp.tile([P, c.FW], F32, space="PSUM", tag="h")
                        nc.tensor.matmul(
                            h_ps[:], lhsT=xT_sb[:], rhs=wcat[:], start=True,
                            stop=True,
                        )
                        hn = sp.tile([P, c.FW], BF16, tag="hn")
                        nc.vector.tensor_scalar(
                            out=hn[:], in0=h_ps[:],
                            scalar1=dinv_sb[:, t : t + 1],
                            scalar2=None, op0=ALU.mult,
                        )
                        nc.sync.dma_start(
                            hn_local[t * P : (t + 1) * P, :], hn[:]
                        )

                    r_lo = int(c.QSTART[k]) * P
                    nc.gpsimd.collective_compute(
                        "AllGather",
                        ALU.bypass,
                        ins=[hn_local[r_lo : r_lo + c.QROWS[k], :]],
                        outs=[hn_q[k][:]],
                        replica_groups=groups,
                    )

            # ---- P4: gather + scatter-add + feat + pooling ----
            # Bank-round order within tile groups: gathers for bank k are
            # issued together so the Pool engine only ever waits for
            # AllGather k (never queues behind a later bank's AllGather).
            # Per-tile PSUM accumulators live across the 4 bank passes.
            GT = 6  # tiles per group: PSUM is bank-granular — 6 accs + 2 pool
            with (
                tc.tile_pool(name="gat", bufs=8) as gp,
                tc.tile_pool(name="psacc", bufs=1, space="PSUM") as pa,
            ):
                pool_ps0 = pa.tile([P, c.FEAT], F32, space="PSUM")
                pool_ps1 = pa.tile([P, c.FEAT], F32, space="PSUM")
                n_gather = 0
                accs = {}
                first_bank = {}
                for t in range(c.T):
                    banks = [j for j in range(c.BANKS) if Bmat[t, j]]
                    first_bank[t] = banks[0] if banks else -1
                for g0 in range(0, c.T, GT):
                    tiles = range(g0, min(g0 + GT, c.T))
                    for t in tiles:
                        accs[t] = pa.tile([P, c.FW], F32, space="PSUM",
                                          tag=f"acc{t - g0}",
                                          name=f"acc_g{g0}_{t - g0}")
                    for j in range(c.BANKS):
                        for t in tiles:
                            B = int(Bmat[t, j])
                            if B == 0:
                                continue
                            cb = dl_col[(t, j)]
                            gt_t = gp.tile([P, B * P], BF16, tag="gt")
                            if n_gather < 8:
                                # -1 slots leave SBUF untouched; scrub the
                                # first use of each pool slot so no NaN
                                # garbage reaches the matmul inputs
                                nc.vector.memset(gt_t[:], 0.0)
                            gi = cell_seq[(t, j)]
                            reg = regs[n_gather % 2]
                            nc.gpsimd.reg_load(reg, nreal_sb[0:1, gi : gi + 1])
                            nc.gpsimd.dma_gather(
                                gt_t[:].rearrange("p (b e) -> p b e", e=P),
                                hn_q[j][:],
                                eidx_sb[:, cb * 8 : cb * 8 + B * 8],
                                B * P,
                                reg,
                                c.FW,
                                single_packet=(B * P <= 1024),
                                queue_num=n_gather % 4,
                            )
                            n_gather += 1
                            oh = op_.tile([P, B * P], BF16, tag="ohb2")
                            nc.vector.tensor_tensor(
                                out=oh[:].rearrange("p (g d) -> p g d", d=P),
                                in0=iota_sb[:].unsqueeze(1).broadcast_to(
                                    [P, B, P]
                                ),
                                in1=dl_sb[:, cb : cb + B].to_broadcast(
                                    [P, B, P]
                                ),
                                op=ALU.is_equal,
                            )
                            for q in range(B):
                                nc.tensor.matmul(
                                    accs[t][:],
                                    lhsT=oh[:, q * P : (q + 1) * P],
                                    rhs=gt_t[:, q * P : (q + 1) * P],
                                    start=(j == first_bank[t] and q == 0),
                                    stop=False,
                                )
                    for t in tiles:
                        acc = accs[t]
                        hno = sp.tile([P, c.FW], BF16, tag="hno")
                        nc.sync.dma_start(
                            hno[:], hn_local[t * P : (t + 1) * P, :]
                        )
                        nc.tensor.matmul(
                            acc[:], lhsT=ident_sb[:], rhs=hno[:],
                            start=(first_bank[t] < 0), stop=True,
                        )

                        ot = sp.tile([P, c.FW], F32, tag="ot")
                        nc.scalar.activation(
                            ot[:], acc[:], AF.Copy, scale=dinv_sb[:, t : t + 1]
                        )
                        nc.vector.tensor_tensor(
                            out=ot[:], in0=ot[:], in1=bias_sb[:], op=ALU.add
                        )
                        feat = sp.tile([P, c.FEAT], BF16, tag="feat")
                        nc.scalar.activation(feat[:, 0:H], ot[:, 0:H], AF.Relu)
                        nc.scalar.copy(feat[:, H : 2 * H], ot[:, 0:H])
                        nc.scalar.activation(
                            feat[:, 2 * H : 3 * H], ot[:, H : 2 * H], AF.Relu
                        )
                        nc.scalar.copy(feat[:, 3 * H : 4 * H], ot[:, H : 2 * H])

                        ohg = sp.tile([P, 2 * P], BF16, tag="ohg")
                        nc.vector.tensor_scalar(
                            out=ohg[:], in0=iota256_sb[:],
                            scalar1=batch_sb[:, t : t + 1], scalar2=None,
                            op0=ALU.is_equal,
                        )
                        nc.tensor.matmul(
                            pool_ps0[:], lhsT=ohg[:, 0:P], rhs=feat[:],
                            start=(t == 0), stop=(t == c.T - 1),
                        )
                        nc.tensor.matmul(
                            pool_ps1[:], lhsT=ohg[:, P : 2 * P], rhs=feat[:],
                            start=(t == 0), stop=(t == c.T - 1),
                        )

                # ---- P5: write local pooled window (zero guard rows on
                # both sides so the combine can read aligned 128-row blocks)
                zt = sp.tile([P, c.FEAT], BF16, tag="zt")
                nc.vector.memset(zt[:], 0.0)
                nc.sync.dma_start(pwin[0:P, :], zt[:])
                nc.sync.dma_start(pwin[3 * P : 4 * P, :], zt[:])
                pp0 = sp.tile([P, c.FEAT], BF16, tag="pp0")
                nc.vector.tensor_copy(pp0[:], pool_ps0[:])
                nc.sync.dma_start(pwin[P : 2 * P, :], pp0[:])
                pp1 = sp.tile([P, c.FEAT], BF16, tag="pp1")
                nc.vector.tensor_copy(pp1[:], pool_ps1[:])
                nc.sync.dma_start(pwin[2 * P : 3 * P, :], pp1[:])

            # ---- P6: AllGather pooled windows, combine statically ----
            nc.gpsimd.collective_compute(
                "AllGather",
                ALU.bypass,
                ins=[pwin[:]],
                outs=[pall[:]],
                replica_groups=groups,
            )

            # ---- P7: mean, FC, log_softmax (replicated) ----
            with tc.tile_pool(name="ps7", bufs=2, space="PSUM") as pp:
                g_base = meta["g_base"]
                for b in range(c.GB):
                    h_rows = min(P, c.G - b * P)
                    tt = sp.tile([P, c.FEAT], F32, tag="tt")
                    nc.vector.memset(tt[:], 0.0)
                    for cc in range(c.NC):
                        d = b * P - int(g_base[cc])
                        if d <= -P or d >= 2 * P:
                            continue  # no overlap with this core's window
                        stg = sp.tile([P, c.FEAT], BF16, tag="stg")
                        r0 = cc * 4 * P + P + d  # guard rows make this valid
                        nc.sync.dma_start(stg[:], pall[r0 : r0 + P, :])
                        nc.vector.tensor_tensor(
                            out=tt[:], in0=tt[:], in1=stg[:], op=ALU.add
                        )
                    mean_sb = sp.tile([P, 4 * H], F32, tag="mean")
                    nc.vector.tensor_scalar(
                        out=mean_sb[:], in0=tt[:, 0 : 4 * H],
                        scalar1=cinv_sb[:, b : b + 1], scalar2=None,
                        op0=ALU.mult,
                    )
                    lg_ps = pp.tile([P, P], F32, space="PSUM", tag="lg")
                    for half in range(2):
                        tp_ps = pp.tile([P, P], F32, space="PSUM", tag="tp")
                        nc.tensor.transpose(
                            tp_ps[:], mean_sb[:, half * P : (half + 1) * P],
                            ident32_sb[:],
                        )
                        mt = sp.tile([P, P], F32, tag="mt")
                        nc.vector.tensor_copy(mt[:], tp_ps[:])
                        nc.tensor.matmul(
                            lg_ps[0 : c.OUT_F, :],
                            lhsT=(fw0 if half == 0 else fw1)[:],
                            rhs=mt[:],
                            start=(half == 0),
                            stop=(half == 1),
                        )
                    lgb = sp.tile([c.OUT_F, P], F32, tag="lgb")
                    nc.vector.tensor_scalar(
                        out=lgb[:], in0=lg_ps[0 : c.OUT_F, :],
                        scalar1=fcb[:, 0:1], scalar2=None, op0=ALU.add,
                    )
                    tr_ps = pp.tile([P, c.OUT_F], F32, space="PSUM", tag="tr")
                    nc.tensor.transpose(
                        tr_ps[:], lgb[:], ident32_sb[0 : c.OUT_F, 0 : c.OUT_F]
                    )
                    ls = sp.tile([P, c.OUT_F], F32, tag="ls")
                    nc.vector.tensor_copy(ls[:], tr_ps[:])
                    mx = sp.tile([P, 1], F32, tag="mx")
                    nc.vector.reduce_max(mx[:], ls[:], axis=mybir.AxisListType.X)
                    nc.vector.tensor_scalar(
                        out=ls[:], in0=ls[:], scalar1=mx[:, 0:1], scalar2=None,
                        op0=ALU.subtract,
                    )
                    ex = sp.tile([P, c.OUT_F], F32, tag="ex")
                    nc.scalar.activation(ex[:], ls[:], AF.Exp)
                    sm = sp.tile([P, 1], F32, tag="sm")
                    nc.vector.reduce_sum(sm[:], ex[:], axis=mybir.AxisListType.X)
                    nc.scalar.activation(sm[:], sm[:], AF.Ln)
                    nc.vector.tensor_scalar(
                        out=ls[:], in0=ls[:], scalar1=sm[:, 0:1], scalar2=None,
                        op0=ALU.subtract,
                    )
                    nc.sync.dma_start(
                        out[b * P : b * P + h_rows, :], ls[0:h_rows, :]
                    )

    nc.compile()
    return nc


def make_in_maps(cfg, meta, per_core, W_td, b_td, W_bu, b_bu, fc_W, fc_b):
    cst = meta["consts"]
    in_maps = []
    for cc in range(cfg.NC):
        pc = per_core[cc]
        in_maps.append(
            {
                "xT": pc["xT"],
                "dinvT": pc["dinvT"],
                "cinvT": pc["cinvT"],
                "W_td": np.asarray(W_td, dtype=np.float32),
                "W_bu": np.asarray(W_bu, dtype=np.float32),
                "b_td": np.asarray(b_td, dtype=np.float32),
                "b_bu": np.asarray(b_bu, dtype=np.float32),
                "fc_W": np.asarray(fc_W, dtype=np.float32),
                "fc_b": np.asarray(fc_b, dtype=np.float32),
                "eidx": pc["eidx"],
                "dlh": pc["dlh"],
                "batchT": pc["batchT"],
                "goff0": pc["goff0"],
                "goff1": pc["goff1"],
                "nreal": pc["nreal"],
                "iota_r": cst["iota_r"],
                "iota256": cst["iota256"],
                "ident": cst["ident"],
            }
        )
    return in_maps


def prep_and_build(cfg, inputs, debug=False):
    x = np.asarray(inputs["x"], dtype=np.float32)
    edge_index = np.asarray(inputs["edge_index"])
    batch = np.asarray(inputs["batch"]).astype(np.int64)
    meta, per_core = host_prep(cfg, x, edge_index, batch)
    nc = build_program(cfg, meta, debug=debug)
    in_maps = make_in_maps(
        cfg, meta, per_core,
        inputs["W_td"], inputs["b_td"], inputs["W_bu"], inputs["b_bu"],
        inputs["fc_W"], inputs["fc_b"],
    )
    return nc, in_maps


def run(cfg, inputs, debug=False, trace=False):
    nc, in_maps = prep_and_build(cfg, inputs, debug=debug)
    res = run_bass_kernel_spmd(nc, in_maps, list(range(cfg.NC)), trace=trace)
    out = res.results[0]["out"].astype(np.float32)
    return out, res


def full_cfg():
    return Cfg(
        n_nodes=100000, n_graphs=1000, n_cores=8, banks=4,
        in_f=128, hid_f=64, out_f=4,
    )


def kernel(**inputs):
    out, _ = run(full_cfg(), inputs)
    return out



# revision 28
# speedup vs baseline: 2.9709x; 1.0559x over previous
"""BiGCN (two fused GCNConv + graph mean-pool + FC + log_softmax) on 8 trn2 cores.

Strategy (graph/data parallel, partitioned by destination node range):
  - core c owns nodes [c*NSH, (c+1)*NSH) as edge destinations
  - host sorts edges into per-(dst-tile, table-bank) cells, padded to 128-slot
    chunks with -1 indices (skipped by the gather HW); per-core real counts
    are fed through gpsimd registers so pad slots cost no DMA descriptors
  - the bf16 Hn table is built as 4 bank stripes, each AllGathered separately
    so bank-0 gathers start while later banks are still being produced
  - host precomputes dinv = 1/sqrt(deg+1), graph-size reciprocals, and the
    bf16-transposed x shard, so the device does no degree work
  - device: Hn = (xT.T @ [W_td | W_bu]) * dinv  (bf16), 4x AllGather -> banks
            dma_gather Hn[src] rows per cell in BANK-ROUND order over tile
            groups (6 PSUM accumulators live across the 4 bank passes) so the
            Pool engine only ever waits on the bank it is gathering from;
            one-hot matmul scatter into the group's PSUM accs
            out[d] = dinv[d]*(sum + Hn[d]) + b ; feat = [relu(td),td,relu(bu),bu]
            graph pooling via one-hot matmul into guarded bf16 windows,
            one small AllGather + static window combine, then
            FC + log_softmax computed replicated on every core.
  - one-hot matrices are built in one DVE tensor_tensor per cell using
    broadcast access patterns (iota row vs per-slot dst-local values).
"""

import math

import numpy as np
import ml_dtypes

import concourse.bass as bass
import concourse.bacc as bacc
import concourse.mybir as mybir
import concourse.tile as tile
from concourse.bass import IndirectOffsetOnAxis
from concourse.bass_utils import run_bass_kernel_spmd
from concourse.library_config import mlp as mlp_lib

BF16 = mybir.dt.bfloat16
F32 = mybir.dt.float32
I16 = mybir.dt.int16
I32 = mybir.dt.int32
AF = mybir.ActivationFunctionType
ALU = mybir.AluOpType
NPBF = ml_dtypes.bfloat16

P = 128  # partitions / tile height


def _split_even(n, k):
    base = n // k
    rem = n % k
    return [base + (1 if i < rem else 0) for i in range(k)]


class Cfg:
    def __init__(self, n_nodes, n_graphs, n_cores, banks, in_f, hid_f, out_f):
        assert n_nodes % n_cores == 0
        self.N = n_nodes
        self.G = n_graphs
        self.NC = n_cores
        self.NSH = n_nodes // n_cores  # nodes per core
        self.T = math.ceil(self.NSH / P)  # dst tiles per core
        self.NSH_P = self.T * P  # padded shard rows
        self.BANKS = min(banks, self.T)
        # bank k holds the stripe of tiles [qt_start[k], qt_start[k+1]) from
        # every core: bank rows = NC * qrows[k]
        self.QT = _split_even(self.T, self.BANKS)  # tiles per bank stripe
        self.QSTART = np.concatenate([[0], np.cumsum(self.QT)])  # tile starts
        self.QROWS = [q * P for q in self.QT]
        for k in range(self.BANKS):
            assert self.NC * self.QROWS[k] <= 32767, "bank idx must fit int16"
        self.IN_F = in_f
        self.HID = hid_f
        self.FW = 2 * hid_f
        assert self.FW == P and in_f == P
        self.OUT_F = out_f
        self.FEAT = 4 * hid_f
        self.GB = math.ceil(self.G / P)
        self.PART_ROWS = (self.G + 2 * P + P - 1) // P * P


def host_prep(cfg, x, edge_index, batch):
    """Build per-core edge grids + constants. Returns (meta, per_core_inputs)."""
    c = cfg
    src = edge_index[0].astype(np.int64)
    dst = edge_index[1].astype(np.int64)
    assert src.min() >= 0 and src.max() < c.N and dst.min() >= 0 and dst.max() < c.N

    # host-side degree normalization (in-degree incl self-loop)
    deg = np.bincount(dst, minlength=c.N).astype(np.float64) + 1.0
    dinv_all = (1.0 / np.sqrt(deg)).astype(np.float32)

    # host-side graph-size reciprocals for mean pooling
    cnt = np.maximum(np.bincount(np.asarray(batch), minlength=c.G), 1)
    cinv = np.zeros(c.GB * P, dtype=np.float32)
    cinv[: c.G] = 1.0 / cnt.astype(np.float64)
    cinvT = cinv.reshape(c.GB, P).T.copy()  # [P, GB]

    qstart_rows = c.QSTART[:-1] * P  # local row where each bank stripe starts
    sc = src // c.NSH  # owner core of src
    so = src % c.NSH  # local row of src
    stile = so // P
    bank = np.searchsorted(c.QSTART[1:], stile, side="right")
    lidx = sc * np.asarray(c.QROWS)[bank] + (so - qstart_rows[bank])

    owner = dst // c.NSH
    tloc = (dst % c.NSH) // P
    dl = ((dst % c.NSH) % P).astype(np.int64)

    ncell = c.NC * c.T * c.BANKS
    cell = (owner * c.T + tloc) * c.BANKS + bank
    order = np.argsort(cell, kind="stable")
    cell_s = cell[order]
    lidx_s = lidx[order]
    dl_s = dl[order]
    counts = np.bincount(cell_s, minlength=ncell).reshape(c.NC, c.T, c.BANKS)
    starts = np.zeros(ncell + 1, dtype=np.int64)
    np.cumsum(counts.reshape(-1), out=starts[1:])

    Bmat = (-(-counts // P)).max(axis=0)  # [T, BANKS] chunks per cell
    nmax16 = -(-np.maximum(counts.max(axis=0), 1) // 16) * 16  # [T, BANKS]
    GCH = int(Bmat.sum())
    ECOLS = GCH * 8

    g_base = np.empty(c.NC, dtype=np.int64)
    for cc in range(c.NC):
        b = batch[cc * c.NSH : (cc + 1) * c.NSH]
        g_base[cc] = int(b[0])
        assert int(b[-1]) - int(b[0]) < 2 * P, "graph span exceeds 2 blocks"

    # chunk columns ordered (t, j, q); shared by dl, eidx and the gather seq
    dl_col = {}
    col = 0
    cells = []  # active cells in order
    for t in range(c.T):
        for j in range(c.BANKS):
            dl_col[(t, j)] = col
            if Bmat[t, j]:
                cells.append((t, j))
            col += int(Bmat[t, j])
    NG = max(len(cells), 1)

    per_core = []
    for cc in range(c.NC):
        eidx = np.zeros((P, max(ECOLS, 8)), dtype=np.int16)
        dlh = np.full((P, max(GCH, 1)), 200.0, dtype=np.float32)
        nreal = np.zeros((1, NG), dtype=np.int32)
        for gi, (t, j) in enumerate(cells):
            B = int(Bmat[t, j])
            ci = (cc * c.T + t) * c.BANKS + j
            s0, s1 = starts[ci], starts[ci + 1]
            n = int(s1 - s0)
            slots = B * P
            li = np.full(slots, -1, dtype=np.int64)
            dv = np.full(slots, 200.0, dtype=np.float64)
            li[:n] = lidx_s[s0:s1]
            dv[:n] = dl_s[s0:s1]
            if n == 0:
                li[0] = 0  # keep >=1 real idx (sim/ucode edge case)
                n = 1
            nreal[0, gi] = n
            w = li.reshape(slots // 16, 16).T.astype(np.int16)
            cb = dl_col[(t, j)]
            eidx[:, cb * 8 : cb * 8 + B * 8] = np.tile(w, (8, 1))
            dlh[:, cb : cb + B] = dv.reshape(B, P).T.astype(np.float32)

        xs = np.zeros((c.NSH_P, c.IN_F), dtype=np.float32)
        xs[: c.NSH] = x[cc * c.NSH : (cc + 1) * c.NSH]
        xT = np.ascontiguousarray(xs.T).astype(NPBF)  # [IN_F, NSH_P]

        dinv_pc = np.ones(c.NSH_P, dtype=np.float32)
        dinv_pc[: c.NSH] = dinv_all[cc * c.NSH : (cc + 1) * c.NSH]
        dinvT = dinv_pc.reshape(c.T, P).T.copy()  # [P, T]

        brel = np.full(c.T * P, 60000.0, dtype=np.float32)
        brel[: c.NSH] = batch[cc * c.NSH : (cc + 1) * c.NSH] - g_base[cc]
        batchT = brel.reshape(c.T, P).T.astype(np.float32)

        goff0 = (g_base[cc] + np.arange(P)).astype(np.int32).reshape(P, 1)
        goff1 = goff0 + P
        per_core.append(
            dict(xT=xT, dinvT=dinvT, cinvT=cinvT, eidx=eidx,
                 dlh=dlh.astype(NPBF), batchT=batchT, goff0=goff0,
                 goff1=goff1, nreal=nreal)
        )

    iota_r = np.tile(np.arange(P, dtype=np.float32), (P, 1)).astype(NPBF)
    iota256 = np.tile(np.arange(2 * P, dtype=np.float32), (P, 1)).astype(NPBF)
    ident = np.eye(P, dtype=np.float32).astype(NPBF)
    consts = dict(iota_r=iota_r, iota256=iota256, ident=ident)
    meta = dict(Bmat=Bmat, GCH=GCH, ECOLS=max(ECOLS, 8), consts=consts,
                dl_col=dl_col, cells=cells, NG=NG, g_base=g_base,
                nmax16=nmax16)
    return meta, per_core


def build_program(cfg, meta, debug=False):
    c = cfg
    Bmat = meta["Bmat"]
    GCH = meta["GCH"]
    ECOLS = meta["ECOLS"]
    dl_col = meta["dl_col"]
    cells = meta["cells"]
    nmax16 = meta["nmax16"]
    NG = meta["NG"]
    H = c.HID
    cell_seq = {tj: gi for gi, tj in enumerate(cells)}

    nc = bacc.Bacc(
        "TRN2", target_bir_lowering=False, debug=debug, num_devices=c.NC,
        num_swdge_queues=4,
    )

    # ---- I/O ----
    xT_in = nc.dram_tensor("xT", [c.IN_F, c.NSH_P], BF16, kind="ExternalInput")
    dinv_in = nc.dram_tensor("dinvT", [P, c.T], F32, kind="ExternalInput")
    cinv_in = nc.dram_tensor("cinvT", [P, c.GB], F32, kind="ExternalInput")
    W_td = nc.dram_tensor("W_td", [c.IN_F, H], F32, kind="ExternalInput")
    W_bu = nc.dram_tensor("W_bu", [c.IN_F, H], F32, kind="ExternalInput")
    b_td = nc.dram_tensor("b_td", [H], F32, kind="ExternalInput")
    b_bu = nc.dram_tensor("b_bu", [H], F32, kind="ExternalInput")
    fc_W = nc.dram_tensor("fc_W", [4 * H, c.OUT_F], F32, kind="ExternalInput")
    fc_b = nc.dram_tensor("fc_b", [c.OUT_F], F32, kind="ExternalInput")
    eidx = nc.dram_tensor("eidx", [P, ECOLS], I16, kind="ExternalInput")
    dlh = nc.dram_tensor("dlh", [P, max(GCH, 1)], BF16, kind="ExternalInput")
    batchT = nc.dram_tensor("batchT", [P, c.T], F32, kind="ExternalInput")
    goff0 = nc.dram_tensor("goff0", [P, 1], I32, kind="ExternalInput")
    goff1 = nc.dram_tensor("goff1", [P, 1], I32, kind="ExternalInput")
    nreal = nc.dram_tensor("nreal", [1, NG], I32, kind="ExternalInput")
    iota_r = nc.dram_tensor("iota_r", [P, P], BF16, kind="ExternalInput")
    iota256_in = nc.dram_tensor("iota256", [P, 2 * P], BF16, kind="ExternalInput")
    ident_in = nc.dram_tensor("ident", [P, P], BF16, kind="ExternalInput")
    out = nc.dram_tensor("out", [c.G, c.OUT_F], F32, kind="ExternalOutput")

    # ---- internal DRAM ----
    hn_local = nc.dram_tensor("hn_local", [c.NSH_P, c.FW], BF16)
    hn_q = [
        nc.dram_tensor(f"hn_q{k}", [c.NC * c.QROWS[k], c.FW], BF16,
                       addr_space="Shared")
        for k in range(c.BANKS)
    ]
    pwin = nc.dram_tensor("pwin", [4 * P, c.FEAT], BF16)
    pall = nc.dram_tensor("pall", [c.NC * 4 * P, c.FEAT], BF16,
                          addr_space="Shared")

    groups = [list(range(c.NC))]

    with tile.TileContext(nc) as tc:
        with (
            tc.tile_pool(name="const", bufs=1) as cp,
            tc.tile_pool(name="sb", bufs=3) as sp,
            tc.tile_pool(name="ohb", bufs=2) as op_,
            nc.gpsimd.register("nr0") as r0,
            nc.gpsimd.register("nr1") as r1,
        ):
            regs = [r0, r1]
            nc.gpsimd.load_library(mlp_lib)

            # ---- constants ----
            iota_sb = cp.tile([P, P], BF16)
            iota256_sb = cp.tile([P, 2 * P], BF16)
            ident_sb = cp.tile([P, P], BF16)
            nc.sync.dma_start(iota_sb[:], iota_r[:])
            nc.sync.dma_start(iota256_sb[:], iota256_in[:])
            nc.sync.dma_start(ident_sb[:], ident_in[:])
            ident32_sb = cp.tile([P, P], F32)
            nc.scalar.activation(ident32_sb[:], ident_sb[:], AF.Copy)

            wcat = cp.tile([P, c.FW], BF16)
            nc.gpsimd.dma_start(wcat[:, 0:H], W_td[:])
            nc.gpsimd.dma_start(wcat[:, H : 2 * H], W_bu[:])

            ones_bf = cp.tile([P, 1], BF16)
            nc.vector.memset(ones_bf[:], 1.0)

            bcat = cp.tile([1, c.FW], BF16)
            nc.gpsimd.dma_start(bcat[0:1, 0:H], b_td[None, :])
            nc.gpsimd.dma_start(bcat[0:1, H : 2 * H], b_bu[None, :])
            ones_row = cp.tile([1, P], BF16)
            nc.vector.memset(ones_row[:], 1.0)
            bias_sb = cp.tile([P, c.FW], F32)

            fw0 = cp.tile([P, c.OUT_F], F32)
            fw1 = cp.tile([P, c.OUT_F], F32)
            nc.sync.dma_start(fw0[:], fc_W[0:P, :])
            nc.sync.dma_start(fw1[:], fc_W[P : 2 * P, :])
            fcb = cp.tile([c.OUT_F, 1], F32)
            nc.sync.dma_start(fcb[:, 0:1], fc_b[:, None])

            eidx_sb = cp.tile([P, ECOLS], I16)
            nc.sync.dma_start(eidx_sb[:], eidx[:])
            dl_sb = cp.tile([P, max(GCH, 1)], BF16)
            nc.sync.dma_start(dl_sb[:], dlh[:])
            batch_sb = cp.tile([P, c.T], F32)
            nc.sync.dma_start(batch_sb[:], batchT[:])
            goff0_sb = cp.tile([P, 1], I32)
            goff1_sb = cp.tile([P, 1], I32)
            nc.sync.dma_start(goff0_sb[:], goff0[:])
            nc.sync.dma_start(goff1_sb[:], goff1[:])
            nreal_sb = cp.tile([1, NG], I32)
            nc.sync.dma_start(nreal_sb[:], nreal[:])

            dinv_sb = cp.tile([P, c.T], F32)
            nc.sync.dma_start(dinv_sb[:], dinv_in[:])
            cinv_sb = cp.tile([P, c.GB], F32)
            nc.sync.dma_start(cinv_sb[:], cinv_in[:])

            def onehot_big(t, tag):
                g0 = dl_col[(t, 0)]
                gt = sum(int(Bmat[t, j]) for j in range(c.BANKS))
                oh = op_.tile([P, gt * P], BF16, tag=tag)
                nc.vector.tensor_tensor(
                    out=oh[:].rearrange("p (g d) -> p g d", d=P),
                    in0=iota_sb[:].unsqueeze(1).broadcast_to([P, gt, P]),
                    in1=dl_sb[:, g0 : g0 + gt].to_broadcast([P, gt, P]),
                    op=ALU.is_equal,
                )
                return oh, g0, gt

            # ---- P1/P2 per bank stripe, AllGather each stripe when ready ----
            with tc.tile_pool(name="ps12", bufs=2, space="PSUM") as pp:
                bias_ps = pp.tile([P, c.FW], F32, space="PSUM", tag="bias")
                nc.tensor.matmul(
                    bias_ps[:], lhsT=ones_row[0:1, :], rhs=bcat[0:1, :],
                    start=True, stop=True,
                )
                nc.vector.tensor_copy(bias_sb[:], bias_ps[:])

                for k in range(c.BANKS):
                    t0k, t1k = int(c.QSTART[k]), int(c.QSTART[k + 1])
                    xT_sb = sp.tile([P, (t1k - t0k) * P], BF16, tag="xTs")
                    nc.sync.dma_start(
                        xT_sb[:], xT_in[:, t0k * P : t1k * P]
                    )
                    for t in range(t0k, t1k):
                        toff = (t - t0k) * P
                        h_ps = pp.tile([P, c.FW], F32, space="PSUM", tag="h")
                        nc.tensor.matmul(
                            h_ps[:], lhsT=xT_sb[:, toff : toff + P],
                            rhs=wcat[:], start=True, stop=True,
                        )
                        hn = sp.tile([P, c.FW], BF16, tag="hn")
                        nc.vector.tensor_scalar(
                            out=hn[:], in0=h_ps[:],
                            scalar1=dinv_sb[:, t : t + 1],
                            scalar2=None, op0=ALU.mult,
                        )
                        nc.sync.dma_start(
                            hn_local[t * P : (t + 1) * P, :], hn[:]
                        )

                    r_lo = int(c.QSTART[k]) * P
                    nc.gpsimd.collective_compute(
                        "AllGather",
                        ALU.bypass,
                        ins=[hn_local[r_lo : r_lo + c.QROWS[k], :]],
                        outs=[hn_q[k][:]],
                        replica_groups=groups,
                    )

            # ---- P4: gather + scatter-add + feat + pooling ----
            # Bank-round order within tile groups: gathers for bank k are
            # issued together so the Pool engine only ever waits for
            # AllGather k (never queues behind a later bank's AllGather).
            # Per-tile PSUM accumulators live across the 4 bank passes.
            GT = 6  # tiles per group: PSUM is bank-granular — 6 accs + 2 pool
            with (
                tc.tile_pool(name="gat", bufs=8) as gp,
                tc.tile_pool(name="psacc", bufs=1, space="PSUM") as pa,
            ):
                pool_ps0 = pa.tile([P, c.FEAT], F32, space="PSUM")
                pool_ps1 = pa.tile([P, c.FEAT], F32, space="PSUM")
                n_gather = 0
                accs = {}
                first_bank = {}
                for t in range(c.T):
                    banks = [j for j in range(c.BANKS) if Bmat[t, j]]
                    first_bank[t] = banks[0] if banks else -1
                for g0 in range(0, c.T, GT):
                    tiles = range(g0, min(g0 + GT, c.T))
                    for t in tiles:
                        accs[t] = pa.tile([P, c.FW], F32, space="PSUM",
                                          tag=f"acc{t - g0}",
                                          name=f"acc_g{g0}_{t - g0}")
                    for j in range(c.BANKS):
                        for t in tiles:
                            B = int(Bmat[t, j])
                            if B == 0:
                                continue
                            cb = dl_col[(t, j)]
                            gt_t = gp.tile([P, B * P], BF16, tag="gt")
                            if n_gather < 8:
                                # -1 slots leave SBUF untouched; scrub the
                                # first use of each pool slot so no NaN
                                # garbage reaches the matmul inputs
                                nc.vector.memset(gt_t[:], 0.0)
                            gi = cell_seq[(t, j)]
                            reg = regs[n_gather % 2]
                            nc.gpsimd.reg_load(reg, nreal_sb[0:1, gi : gi + 1])
                            nc.gpsimd.dma_gather(
                                gt_t[:].rearrange("p (b e) -> p b e", e=P),
                                hn_q[j][:],
                                eidx_sb[:, cb * 8
                                        : cb * 8 + int(nmax16[t, j]) // 16],
                                int(nmax16[t, j]),
                                reg,
                                c.FW,
                                single_packet=(B * P <= 1024),
                                queue_num=n_gather % 4,
                            )
                            n_gather += 1
                            oh = op_.tile([P, B * P], BF16, tag="ohb2")
                            nc.vector.tensor_tensor(
                                out=oh[:].rearrange("p (g d) -> p g d", d=P),
                                in0=iota_sb[:].unsqueeze(1).broadcast_to(
                                    [P, B, P]
                                ),
                                in1=dl_sb[:, cb : cb + B].to_broadcast(
                                    [P, B, P]
                                ),
                                op=ALU.is_equal,
                            )
                            for q in range(B):
                                nc.tensor.matmul(
                                    accs[t][:],
                                    lhsT=oh[:, q * P : (q + 1) * P],
                                    rhs=gt_t[:, q * P : (q + 1) * P],
                                    start=(j == first_bank[t] and q == 0),
                                    stop=False,
                                )
                    for t in tiles:
                        acc = accs[t]
                        hno = sp.tile([P, c.FW], BF16, tag="hno")
                        nc.sync.dma_start(
                            hno[:], hn_local[t * P : (t + 1) * P, :]
                        )
                        nc.tensor.matmul(
                            acc[:], lhsT=ident_sb[:], rhs=hno[:],
                            start=(first_bank[t] < 0), stop=True,
                        )

                        ot = sp.tile([P, c.FW], F32, tag="ot")
                        nc.scalar.activation(
                            ot[:], acc[:], AF.Copy, scale=dinv_sb[:, t : t + 1]
                        )
                        nc.vector.tensor_tensor(
                            out=ot[:], in0=ot[:], in1=bias_sb[:], op=ALU.add
                        )
                        feat = sp.tile([P, c.FEAT], BF16, tag="feat")
                        nc.scalar.activation(feat[:, 0:H], ot[:, 0:H], AF.Relu)
                        nc.scalar.copy(feat[:, H : 2 * H], ot[:, 0:H])
                        nc.scalar.activation(
                            feat[:, 2 * H : 3 * H], ot[:, H : 2 * H], AF.Relu
                        )
                        nc.scalar.copy(feat[:, 3 * H : 4 * H], ot[:, H : 2 * H])

                        ohg = sp.tile([P, 2 * P], BF16, tag="ohg")
                        nc.vector.tensor_scalar(
                            out=ohg[:], in0=iota256_sb[:],
                            scalar1=batch_sb[:, t : t + 1], scalar2=None,
                            op0=ALU.is_equal,
                        )
                        nc.tensor.matmul(
                            pool_ps0[:], lhsT=ohg[:, 0:P], rhs=feat[:],
                            start=(t == 0), stop=(t == c.T - 1),
                        )
                        nc.tensor.matmul(
                            pool_ps1[:], lhsT=ohg[:, P : 2 * P], rhs=feat[:],
                            start=(t == 0), stop=(t == c.T - 1),
                        )

                # ---- P5: write local pooled window (zero guard rows on
                # both sides so the combine can read aligned 128-row blocks)
                zt = sp.tile([P, c.FEAT], BF16, tag="zt")
                nc.vector.memset(zt[:], 0.0)
                nc.sync.dma_start(pwin[0:P, :], zt[:])
                nc.sync.dma_start(pwin[3 * P : 4 * P, :], zt[:])
                pp0 = sp.tile([P, c.FEAT], BF16, tag="pp0")
                nc.vector.tensor_copy(pp0[:], pool_ps0[:])
                nc.sync.dma_start(pwin[P : 2 * P, :], pp0[:])
                pp1 = sp.tile([P, c.FEAT], BF16, tag="pp1")
                nc.vector.tensor_copy(pp1[:], pool_ps1[:])
                nc.sync.dma_start(pwin[2 * P : 3 * P, :], pp1[:])

            # ---- P6: AllGather pooled windows, combine statically ----
            nc.gpsimd.collective_compute(
                "AllGather",
                ALU.bypass,
                ins=[pwin[:]],
                outs=[pall[:]],
                replica_groups=groups,
            )

            # ---- P7: mean, FC, log_softmax (replicated) ----
            with tc.tile_pool(name="ps7", bufs=2, space="PSUM") as pp:
                g_base = meta["g_base"]
                for b in range(c.GB):
                    h_rows = min(P, c.G - b * P)
                    tt = sp.tile([P, c.FEAT], F32, tag="tt")
                    nc.vector.memset(tt[:], 0.0)
                    for cc in range(c.NC):
                        d = b * P - int(g_base[cc])
                        if d <= -P or d >= 2 * P:
                            continue  # no overlap with this core's window
                        stg = sp.tile([P, c.FEAT], BF16, tag="stg")
                        r0 = cc * 4 * P + P + d  # guard rows make this valid
                        nc.sync.dma_start(stg[:], pall[r0 : r0 + P, :])
                        nc.vector.tensor_tensor(
                            out=tt[:], in0=tt[:], in1=stg[:], op=ALU.add
                        )
                    mean_sb = sp.tile([P, 4 * H], F32, tag="mean")
                    nc.vector.tensor_scalar(
                        out=mean_sb[:], in0=tt[:, 0 : 4 * H],
                        scalar1=cinv_sb[:, b : b + 1], scalar2=None,
                        op0=ALU.mult,
                    )
                    lg_ps = pp.tile([P, P], F32, space="PSUM", tag="lg")
                    for half in range(2):
                        tp_ps = pp.tile([P, P], F32, space="PSUM", tag="tp")
                        nc.tensor.transpose(
                            tp_ps[:], mean_sb[:, half * P : (half + 1) * P],
                            ident32_sb[:],
                        )
                        mt = sp.tile([P, P], F32, tag="mt")
                        nc.vector.tensor_copy(mt[:], tp_ps[:])
                        nc.tensor.matmul(
                            lg_ps[0 : c.OUT_F, :],
                            lhsT=(fw0 if half == 0 else fw1)[:],
                            rhs=mt[:],
                            start=(half == 0),
                            stop=(half == 1),
                        )
                    lgb = sp.tile([c.OUT_F, P], F32, tag="lgb")
                    nc.vector.tensor_scalar(
                        out=lgb[:], in0=lg_ps[0 : c.OUT_F, :],
                        scalar1=fcb[:, 0:1], scalar2=None, op0=ALU.add,
                    )
                    tr_ps = pp.tile([P, c.OUT_F], F32, space="PSUM", tag="tr")
                    nc.tensor.transpose(
                        tr_ps[:], lgb[:], ident32_sb[0 : c.OUT_F, 0 : c.OUT_F]
                    )
                    ls = sp.tile([P, c.OUT_F], F32, tag="ls")
                    nc.vector.tensor_copy(ls[:], tr_ps[:])
                    mx = sp.tile([P, 1], F32, tag="mx")
                    nc.vector.reduce_max(mx[:], ls[:], axis=mybir.AxisListType.X)
                    nc.vector.tensor_scalar(
                        out=ls[:], in0=ls[:], scalar1=mx[:, 0:1], scalar2=None,
                        op0=ALU.subtract,
                    )
                    ex = sp.tile([P, c.OUT_F], F32, tag="ex")
                    nc.scalar.activation(ex[:], ls[:], AF.Exp)
                    sm = sp.tile([P, 1], F32, tag="sm")
                    nc.vector.reduce_sum(sm[:], ex[:], axis=mybir.AxisListType.X)
                    nc.scalar.activation(sm[:], sm[:], AF.Ln)
                    nc.vector.tensor_scalar(
                        out=ls[:], in0=ls[:], scalar1=sm[:, 0:1], scalar2=None,
                        op0=ALU.subtract,
                    )
                    nc.sync.dma_start(
                        out[b * P : b * P + h_rows, :], ls[0:h_rows, :]
                    )

    nc.compile()
    return nc


def make_in_maps(cfg, meta, per_core, W_td, b_td, W_bu, b_bu, fc_W, fc_b):
    cst = meta["consts"]
    in_maps = []
    for cc in range(cfg.NC):
        pc = per_core[cc]
        in_maps.append(
            {
                "xT": pc["xT"],
                "dinvT": pc["dinvT"],
                "cinvT": pc["cinvT"],
                "W_td": np.asarray(W_td, dtype=np.float32),
                "W_bu": np.asarray(W_bu, dtype=np.float32),
                "b_td": np.asarray(b_td, dtype=np.float32),
                "b_bu": np.asarray(b_bu, dtype=np.float32),
                "fc_W": np.asarray(fc_W, dtype=np.float32),
                "fc_b": np.asarray(fc_b, dtype=np.float32),
                "eidx": pc["eidx"],
                "dlh": pc["dlh"],
                "batchT": pc["batchT"],
                "goff0": pc["goff0"],
                "goff1": pc["goff1"],
                "nreal": pc["nreal"],
                "iota_r": cst["iota_r"],
                "iota256": cst["iota256"],
                "ident": cst["ident"],
            }
        )
    return in_maps


def prep_and_build(cfg, inputs, debug=False):
    x = np.asarray(inputs["x"], dtype=np.float32)
    edge_index = np.asarray(inputs["edge_index"])
    batch = np.asarray(inputs["batch"]).astype(np.int64)
    meta, per_core = host_prep(cfg, x, edge_index, batch)
    nc = build_program(cfg, meta, debug=debug)
    in_maps = make_in_maps(
        cfg, meta, per_core,
        inputs["W_td"], inputs["b_td"], inputs["W_bu"], inputs["b_bu"],
        inputs["fc_W"], inputs["fc_b"],
    )
    return nc, in_maps


def run(cfg, inputs, debug=False, trace=False):
    nc, in_maps = prep_and_build(cfg, inputs, debug=debug)
    res = run_bass_kernel_spmd(nc, in_maps, list(range(cfg.NC)), trace=trace)
    out = res.results[0]["out"].astype(np.float32)
    return out, res


def full_cfg():
    return Cfg(
        n_nodes=100000, n_graphs=1000, n_cores=8, banks=4,
        in_f=128, hid_f=64, out_f=4,
    )


def kernel(**inputs):
    out, _ = run(full_cfg(), inputs)
    return out

